# revision 4
# baseline (speedup 1.0000x reference)
"""GQA attention kernel for 8 TRN2 NeuronCores.

Problem: B=2, T=2048, C=4096, NH=32 q-heads, NKV=8 kv-heads, HD=128,
RoPE (theta=1e4), causal, f32 I/O.

Sharding: core = (batch b, kv-head-group g): b = core//4, g = core%4.
Each core owns batch b and kv heads {2g, 2g+1} (= q heads 8g..8g+7):
  - projects x[b] against its wq/wk/wv column slices (bf16 compute),
  - runs causal attention for its 8 q heads,
  - computes the partial o_proj x its wo row slice -> [T, C] f32.
Host sums the 4 partials per batch.

On-chip layout is feature-major ("X^T"): activations live as
[feature=partition, token=free] so every matmul contracts along
partitions. x is pre-transposed/bf16-cast on host; RoPE's rotate_half
is a 128x128 permutation matmul on the PE.

Attention uses a "flipped" PV: the P^T tiles produced by the score
matmuls stream through the PE against a stationary V-natural tile,
accumulating O^T (feature-major) directly in PSUM -- 512 columns per
weight load (vs 129 natural, which is LDWEIGHTS-bound) and no
post-attention O transposes. The softmax denominator comes from a
[128,128]-ones matmul over a DVE-accumulated column-sum tile (plus the
last P^T tile directly), already broadcast over partitions; a fast
approximate reciprocal (custom DVE op) and one multiply normalize
O^T on its way out of PSUM. The causal mask is a {0,1} multiply on
the otherwise-idle GpSimd engine, which also carries two of the three
RoPE elementwise ops.
"""

import sys

sys.path.insert(0, "/opt/trn_rl_repo")

import numpy as np
import ml_dtypes

import concourse.bass as bass
import concourse.bacc as bacc
import concourse.mybir as mybir
import concourse.tile as tile
from concourse.bass_utils import run_bass_kernel_spmd

BF16 = mybir.dt.bfloat16
F32 = mybir.dt.float32
AF = mybir.ActivationFunctionType
ALU = mybir.AluOpType

B, T, C = 2, 2048, 4096
NH, NKV, HD = 32, 8, 128
THETA = 10000.0
NCORES = 8

QH = 8          # q heads per core
KV = 2          # kv heads per core
OUTS = 12       # projection output tiles: 8 q + 2 k + 2 v
QC = 4          # token chunks of 512
KT = 16         # k tiles of 128
TT = 16         # token tiles of 128
CCH = 32        # contraction chunks of 128 over C

_CACHE = {}


def _build_nc():
    nc = bacc.Bacc("TRN2", target_bir_lowering=False, debug=False,
                   enable_asserts=False, num_devices=NCORES)

    xT_d = nc.dram_tensor("xT", [C, T], BF16, kind="ExternalInput")
    wqkv_d = nc.dram_tensor("wqkv", [CCH, 2, 128, 768], BF16, kind="ExternalInput")
    wo_d = nc.dram_tensor("wo", [QH * HD, C], BF16, kind="ExternalInput")
    cos_d = nc.dram_tensor("cosT", [128, T], BF16, kind="ExternalInput")
    sin_d = nc.dram_tensor("sinT", [128, T], BF16, kind="ExternalInput")
    prot_d = nc.dram_tensor("protT", [128, 128], BF16, kind="ExternalInput")
    ident_d = nc.dram_tensor("ident", [128, 128], BF16, kind="ExternalInput")
    cmask_d = nc.dram_tensor("cmask01", [128, 4, 128], BF16, kind="ExternalInput")
    out_d = nc.dram_tensor("out", [T, C], F32, kind="ExternalOutput")

    with tile.TileContext(nc) as tc:
        with tc.tile_pool(name="persist", bufs=1) as pp:
            ident = pp.tile([128, 128], BF16)
            nc.sync.dma_start(ident, ident_d.ap())
            cosT = pp.tile([128, T], BF16)
            sinT = pp.tile([128, T], BF16)
            prot = pp.tile([128, 128], BF16)
            cmask = pp.tile([128, 4, 128], BF16)
            ones128 = pp.tile([128, 128], BF16)
            warm = pp.tile([128, 128], BF16)
            nc.vector.memset(ones128, 1.0)
            nc.vector.memset(warm, 0.25)

            # HAM warm-up with REAL matmuls (transposes don't count as
            # PE-busy for the clock gate). No DMA dependency: operand is
            # memset on-chip, so this starts immediately and spans the
            # ~8us until the first x^T block lands, leaving the PE at
            # 2.4 GHz when projections begin.
            with tc.tile_pool(name="pwarm", bufs=2, space="PSUM") as pwp:
                for w in range(96):
                    wps = pwp.tile([128, 128], F32, name=f"warm{w}", tag="warm")
                    nc.tensor.matmul(wps, warm, warm, start=True, stop=True)

            QT = pp.tile([128, QH, T], BF16)
            KTt = pp.tile([128, KV, T], BF16)
            VT = pp.tile([128, KV, T], BF16)
            OT = pp.tile([128, QH, T], BF16)
            Vn = pp.tile([128, KV, KT, 128], BF16)

            # ---------------- projections: Q^T/K^T/V^T = W^T @ x^T ----------
            with tc.tile_pool(name="xt", bufs=2) as xtp, \
                 tc.tile_pool(name="wt", bufs=6) as wtp, \
                 tc.tile_pool(name="pproj", bufs=7, space="PSUM") as ppj:
                xview = xT_d.ap().rearrange("(c p) t -> p c t", p=128)
                for qc in range(QC):
                    tsl = slice(qc * 512, (qc + 1) * 512)
                    xt = xtp.tile([128, CCH, 512], BF16)
                    # split the load (early c-chunks land first) and use the
                    # scalar HWDGE queue so weights stream in parallel on sync
                    for piece in range(4):
                        csl = slice(piece * 8, (piece + 1) * 8)
                        nc.scalar.dma_start(xt[:, csl, :], xview[:, csl, tsl])
                    for grp in range(2):
                        psums = [ppj.tile([128, 512], F32, name=f"pj{qc}_{grp}_{o}",
                                          tag="pj") for o in range(6)]
                        for c in range(CCH):
                            wt = wtp.tile([128, 768], BF16)
                            nc.sync.dma_start(wt, wqkv_d.ap()[c, grp])
                            for o in range(6):
                                nc.tensor.matmul(
                                    psums[o], wt[:, o * 128:(o + 1) * 128],
                                    xt[:, c, :], start=(c == 0), stop=(c == CCH - 1))
                        for o in range(6):
                            oi = grp * 6 + o
                            if oi < 8:
                                dst = QT[:, oi, tsl]
                            elif oi < 10:
                                dst = KTt[:, oi - 8, tsl]
                            else:
                                dst = VT[:, oi - 10, tsl]
                            # alternate engines so psum slots free faster
                            if o % 2 == 0:
                                nc.scalar.copy(dst, psums[o])
                            else:
                                nc.vector.tensor_copy(dst, psums[o])

            # constants for RoPE/attention — loaded once projections are
            # underway so they don't delay the first weight tiles
            nc.scalar.dma_start(cosT, cos_d.ap())
            nc.scalar.dma_start(sinT, sin_d.ap())
            nc.scalar.dma_start(prot, prot_d.ap())
            nc.scalar.dma_start(cmask, cmask_d.ap())

            # wo load after the x^T/weight stream pools are gone, so it
            # overlaps RoPE + attention without blowing SBUF
            wo_pool = tc.alloc_tile_pool(name="wop", bufs=1)
            wo_t = wo_pool.tile([128, QH, C], BF16)
            nc.sync.dma_start(wo_t, wo_d.ap().rearrange("(h p) n -> p h n", p=128))

            # ---------------- attention (with fused RoPE) ------------------
            # rot = P_rot @ q (sign baked into P_rot), q' = q*cos + rot*sin
            # S^T[k,q] = K @ Q^T; P^T = exp(S^T) * {0,1}-mask (gpsimd)
            # O^T[hd,q] = sum_kt V_nat[kt]^T @ P^T[kt]   (flipped PV)
            # denB[*,q] = ones128 @ (sum_kt P^T[kt]); O^T *= approx(1/denB)
            with tc.tile_pool(name="pst", bufs=2, space="PSUM") as pst, \
                 tc.tile_pool(name="pot", bufs=2, space="PSUM") as pot, \
                 tc.tile_pool(name="pdb", bufs=2, space="PSUM") as pdb, \
                 tc.tile_pool(name="pt", bufs=3) as ptp, \
                 tc.tile_pool(name="acc", bufs=2) as accp, \
                 tc.tile_pool(name="rcd", bufs=2) as rcdp, \
                 tc.tile_pool(name="ropes", bufs=3) as rsp:

                def rope(src):
                    for rqc in range(QC):
                        rsl = slice(rqc * 512, (rqc + 1) * 512)
                        ps = pdb.tile([128, 512], F32, name=f"rot{rqc}", tag="db")
                        nc.tensor.matmul(ps, prot, src[:, rsl], start=True,
                                         stop=True)
                        rs = rsp.tile([128, 512], BF16, name=f"rs{rqc}", tag="rs")
                        nc.vector.tensor_tensor(rs, ps, sinT[:, rsl], op=ALU.mult)
                        # the SBUF-only elementwise ops ride on the idle
                        # gpsimd engine; DVE keeps only the PSUM read above
                        nc.gpsimd.tensor_tensor(src[:, rsl], src[:, rsl],
                                                cosT[:, rsl], op=ALU.mult)
                        nc.gpsimd.tensor_tensor(src[:, rsl], src[:, rsl], rs,
                                                op=ALU.add)

                def vtrans(kv):
                    for kt in range(KT):
                        pt = pst.tile([128, 128], BF16, name=f"tv{kv}_{kt}",
                                      tag="st")
                        nc.tensor.transpose(
                            pt, VT[:, kv, kt * 128:(kt + 1) * 128], ident)
                        nc.scalar.copy(Vn[:, kv, kt, :], pt)

                rope(KTt[:, 0, :])
                vtrans(0)
                rope(QT[:, 0, :])
                rope(KTt[:, 1, :])
                vtrans(1)

                for h in range(QH):
                    kv = h // 4
                    if h + 1 < QH:
                        rope(QT[:, h + 1, :])
                    for qc in range(QC):
                        tsl = slice(qc * 512, (qc + 1) * 512)
                        nk = 4 * qc + 4       # k tiles this chunk (even)
                        last_kt = nk - 1
                        otps = pot.tile([128, 512], F32, name=f"ot{h}_{qc}",
                                        tag="ot")
                        acc = accp.tile([128, 512], BF16)

                        # stage 1 of the pair pipeline: score matmuls + exp
                        # (+ gpsimd {0,1} mask for diagonal tiles)
                        def scores(kp, h=h, kv=kv, qc=qc):
                            kt0 = 2 * kp
                            st_pair = pst.tile([128, 2, 512], F32, tag="st")
                            pt_pair = ptp.tile([128, 2, 512], BF16)
                            if kt0 + 1 < 4 * qc:          # full pair
                                for half in (0, 1):
                                    kt = kt0 + half
                                    nc.tensor.matmul(
                                        st_pair[:, half, :],
                                        KTt[:, kv, kt * 128:(kt + 1) * 128],
                                        QT[:, h, qc * 512:(qc + 1) * 512],
                                        start=True, stop=True)
                                # one ACTIVATE covers the whole pair
                                nc.scalar.activation(pt_pair, st_pair, AF.Exp)
                            else:                          # diagonal pair
                                for half in (0, 1):
                                    kt = kt0 + half
                                    d = kt - 4 * qc
                                    vsl = slice(d * 128, 512)
                                    nc.tensor.matmul(
                                        st_pair[:, half, vsl],
                                        KTt[:, kv, kt * 128:(kt + 1) * 128],
                                        QT[:, h,
                                           qc * 512 + d * 128:(qc + 1) * 512],
                                        start=True, stop=True)
                                    nc.scalar.activation(
                                        pt_pair[:, half, vsl],
                                        st_pair[:, half, vsl], AF.Exp)
                                    # zero the above-diagonal part of the
                                    # block straddling the diagonal
                                    nc.gpsimd.tensor_tensor(
                                        pt_pair[:, half,
                                                d * 128:(d + 1) * 128],
                                        pt_pair[:, half,
                                                d * 128:(d + 1) * 128],
                                        cmask[:, d, :], op=ALU.mult)
                            return pt_pair

                        # stage 2: PV accumulation + denominator column-sums
                        def consume(kp, pt_pair, h=h, kv=kv, qc=qc):
                            for half in (0, 1):
                                kt = 2 * kp + half
                                d = kt - 4 * qc
                                vsl = slice(max(d, 0) * 128, 512)
                                ptv = pt_pair[:, half, :]
                                nc.tensor.matmul(
                                    otps[:, vsl], Vn[:, kv, kt, :],
                                    ptv[:, vsl],
                                    start=(kt == 0), stop=(kt == last_kt))
                                if kt == 0 and qc == 0:
                                    nc.vector.tensor_copy(acc, ptv)
                                elif kt == 1 and qc > 0:
                                    nc.vector.tensor_tensor(
                                        acc, pt_pair[:, 0, :],
                                        pt_pair[:, 1, :], op=ALU.add)
                                elif (kt > 1 or qc == 0) and kt < last_kt:
                                    nc.vector.tensor_tensor(
                                        acc[:, vsl], acc[:, vsl], ptv[:, vsl],
                                        op=ALU.add)
                            return pt_pair[:, 1, :]

                        npairs = nk // 2
                        pts = [scores(0)]
                        last_pt = None
                        for kp in range(npairs):
                            if kp + 1 < npairs:
                                pts.append(scores(kp + 1))
                            last_pt = consume(kp, pts[kp])
                        # denominator, pre-broadcast over all 128 partitions
                        denB = pdb.tile([128, 512], F32, tag="db")
                        nc.tensor.matmul(denB, ones128, acc,
                                         start=True, stop=False)
                        nc.tensor.matmul(denB[:, 384:512], ones128,
                                         last_pt[:, 384:512],
                                         start=False, stop=True)
                        rcdB = rcdp.tile([128, 512], F32)
                        nc.vector.reciprocal_approx_fast(rcdB, denB)
                        nc.vector.tensor_tensor(OT[:, h, tsl], otps, rcdB,
                                                op=ALU.mult)

            # ---------------- o_proj partial: O @ wo_slice ----------------
            with tc.tile_pool(name="pout", bufs=6, space="PSUM") as outp, \
                 tc.tile_pool(name="ostg", bufs=6) as stgp:
                for tt in range(TT):
                    psl = slice(tt * 128, (tt + 1) * 128)
                    for n in range(8):
                        nsl = slice(n * 512, (n + 1) * 512)
                        ps = outp.tile([128, 512], F32)
                        for h in range(QH):
                            nc.tensor.matmul(ps, OT[:, h, psl],
                                             wo_t[:, h, nsl],
                                             start=(h == 0), stop=(h == QH - 1))
                        stg = stgp.tile([128, 512], F32)
                        nc.scalar.copy(stg, ps)
                        nc.sync.dma_start(out_d.ap()[psl, nsl], stg)

            wo_pool.release()

    nc.compile()
    return nc


def _host_prep(x, wq, wk, wv, wo):
    bf = ml_dtypes.bfloat16
    scale = HD ** -0.5

    # RoPE tables, feature-major [128, T]
    inv_freq = 1.0 / (THETA ** (np.arange(0, HD, 2, dtype=np.float32) / HD))
    t = np.arange(T, dtype=np.float32)
    freqs = np.outer(t, inv_freq)                      # [T, 64]
    emb = np.concatenate([freqs, freqs], -1)           # [T, 128]
    cosT = np.ascontiguousarray(np.cos(emb).T).astype(bf)
    sinT = np.ascontiguousarray(np.sin(emb).T).astype(bf)

    # rotate_half as a permutation matrix, pre-transposed for lhsT:
    # rot = P_rot @ q with P_rot[i, i+64] = -1 (i<64), P_rot[i, i-64] = +1.
    protT = np.zeros((128, 128), np.float32)
    for i in range(64):
        protT[i + 64, i] = -1.0
        protT[i, i + 64] = 1.0
    protT = protT.astype(bf)

    ident = np.eye(128, dtype=np.float32).astype(bf)

    # {0,1} causal masks for the diagonal [128k, 128q] blocks of the 4
    # diagonal tiles of each 512-q chunk: valid iff q_local >= k_local
    # (block-local indices coincide for every d)
    kl = np.arange(128)[:, None]
    ql = np.arange(128)[None, :]
    blk = (ql >= kl).astype(np.float32)
    cmask01 = np.stack([blk] * 4, axis=1).astype(bf)   # [128, 4, 128]
    cmask01 = np.ascontiguousarray(cmask01)

    xT = []
    for b in range(B):
        xT.append(np.ascontiguousarray(x[b].astype(bf).T))

    wqkv, wob = [], []
    for g in range(4):
        q_s = (wq[:, g * 1024:(g + 1) * 1024] * scale).astype(bf)
        k_s = wk[:, g * 256:(g + 1) * 256].astype(bf)
        v_s = wv[:, g * 256:(g + 1) * 256].astype(bf)
        wall = np.concatenate([q_s, k_s, v_s], axis=1)       # [C, 1536]
        wall = wall.reshape(CCH, 128, 2, 768).transpose(0, 2, 1, 3)
        wqkv.append(np.ascontiguousarray(wall))              # [32, 2, 128, 768]
        wob.append(np.ascontiguousarray(
            wo[g * 1024:(g + 1) * 1024, :].astype(bf)))      # [1024, C]

    in_maps = []
    for core in range(NCORES):
        b, g = core // 4, core % 4
        in_maps.append({
            "xT": xT[b], "wqkv": wqkv[g], "wo": wob[g],
            "cosT": cosT, "sinT": sinT, "protT": protT,
            "ident": ident, "cmask01": cmask01,
        })
    return in_maps


def kernel(x, wq, wk, wv, wo, _trace=False, _tmpdir=None):
    if "nc" not in _CACHE:
        _CACHE["nc"] = _build_nc()
    nc = _CACHE["nc"]

    in_maps = _host_prep(x, wq, wk, wv, wo)
    res = run_bass_kernel_spmd(nc, in_maps, core_ids=list(range(NCORES)),
                               trace=_trace, tmpdir=_tmpdir)
    _CACHE["last_results"] = res

    out = np.zeros((B, T, C), np.float32)
    for core in range(NCORES):
        out[core // 4] += res.results[core]["out"]
    return out


# revision 5
# speedup vs baseline: 1.2379x; 1.2379x over previous
"""GQA attention kernel for 8 TRN2 NeuronCores.

Problem: B=2, T=2048, C=4096, NH=32 q-heads, NKV=8 kv-heads, HD=128,
RoPE (theta=1e4), causal, f32 I/O.

Sharding: core = (batch b, kv-head-group g): b = core//4, g = core%4.
Each core owns batch b and kv heads {2g, 2g+1} (= q heads 8g..8g+7):
  - projects x[b] against its wq/wk/wv column slices (bf16 compute),
  - runs causal attention for its 8 q heads,
  - computes the partial o_proj x its wo row slice -> [T, C] f32.
Host sums the 4 partials per batch.

On-chip layout is feature-major ("X^T"): activations live as
[feature=partition, token=free] so every matmul contracts along
partitions. x is pre-transposed/bf16-cast on host; RoPE's rotate_half
is a 128x128 permutation matmul on the PE; softmax denominator comes
free from a ones-column appended to V.
"""

import sys

sys.path.insert(0, "/opt/trn_rl_repo")

import numpy as np
import ml_dtypes

import concourse.bass as bass
import concourse.bacc as bacc
import concourse.mybir as mybir
import concourse.tile as tile
from concourse.bass_utils import run_bass_kernel_spmd

BF16 = mybir.dt.bfloat16
F32 = mybir.dt.float32
AF = mybir.ActivationFunctionType
ALU = mybir.AluOpType

B, T, C = 2, 2048, 4096
NH, NKV, HD = 32, 8, 128
THETA = 10000.0
NCORES = 8

QH = 8          # q heads per core
KV = 2          # kv heads per core
OUTS = 12       # projection output tiles: 8 q + 2 k + 2 v
QC = 4          # token chunks of 512
KT = 16         # k tiles of 128
TT = 16         # token tiles of 128
CCH = 32        # contraction chunks of 128 over C

_CACHE = {}


def _build_nc():
    nc = bacc.Bacc("TRN2", target_bir_lowering=False, debug=False,
                   enable_asserts=False, num_devices=NCORES)

    xT_d = nc.dram_tensor("xT", [C, T], BF16, kind="ExternalInput")
    wqkv_d = nc.dram_tensor("wqkv", [CCH, 2, 128, 768], BF16, kind="ExternalInput")
    wo_d = nc.dram_tensor("wo", [QH * HD, C], BF16, kind="ExternalInput")
    cos_d = nc.dram_tensor("cosT", [128, T], BF16, kind="ExternalInput")
    sin_d = nc.dram_tensor("sinT", [128, T], BF16, kind="ExternalInput")
    prot_d = nc.dram_tensor("protT", [128, 128], BF16, kind="ExternalInput")
    ident_d = nc.dram_tensor("ident", [128, 128], BF16, kind="ExternalInput")
    cmask_d = nc.dram_tensor("cmask", [128, 4, 512], F32, kind="ExternalInput")
    out_d = nc.dram_tensor("out", [T, C], F32, kind="ExternalOutput")

    with tile.TileContext(nc) as tc:
        with tc.tile_pool(name="persist", bufs=1) as pp:
            ident = pp.tile([128, 128], BF16)
            nc.sync.dma_start(ident, ident_d.ap())
            cosT = pp.tile([128, T], BF16)
            sinT = pp.tile([128, T], BF16)
            prot = pp.tile([128, 128], BF16)
            cmask = pp.tile([128, 4, 512], F32)

            # HAM warm-up: keep the PE busy while the first x^T block
            # DMAs in, so projections start at 2.4 GHz instead of 1.2.
            with tc.tile_pool(name="pwarm", bufs=2, space="PSUM") as pwp:
                for w in range(32):
                    wps = pwp.tile([128, 128], BF16, name=f"warm{w}", tag="warm")
                    nc.tensor.transpose(wps, ident, ident)

            QT = pp.tile([128, QH, T], BF16)
            KTt = pp.tile([128, KV, T], BF16)
            VT = pp.tile([128, KV, T], BF16)
            OT = pp.tile([128, QH, T], BF16)
            Vn = pp.tile([128, KV, KT, 132], BF16)
            nc.vector.memset(Vn[:, :, :, 128:129], 1.0)

            # ---------------- projections: Q^T/K^T/V^T = W^T @ x^T ----------
            with tc.tile_pool(name="xt", bufs=2) as xtp, \
                 tc.tile_pool(name="wt", bufs=6) as wtp, \
                 tc.tile_pool(name="pproj", bufs=7, space="PSUM") as ppj:
                xview = xT_d.ap().rearrange("(c p) t -> p c t", p=128)
                for qc in range(QC):
                    tsl = slice(qc * 512, (qc + 1) * 512)
                    xt = xtp.tile([128, CCH, 512], BF16)
                    # split the load (early c-chunks land first) and use the
                    # scalar HWDGE queue so weights stream in parallel on sync
                    for piece in range(4):
                        csl = slice(piece * 8, (piece + 1) * 8)
                        nc.scalar.dma_start(xt[:, csl, :], xview[:, csl, tsl])
                    for grp in range(2):
                        psums = [ppj.tile([128, 512], F32, name=f"pj{qc}_{grp}_{o}",
                                          tag="pj") for o in range(6)]
                        for c in range(CCH):
                            wt = wtp.tile([128, 768], BF16)
                            nc.sync.dma_start(wt, wqkv_d.ap()[c, grp])
                            for o in range(6):
                                nc.tensor.matmul(
                                    psums[o], wt[:, o * 128:(o + 1) * 128],
                                    xt[:, c, :], start=(c == 0), stop=(c == CCH - 1))
                        for o in range(6):
                            oi = grp * 6 + o
                            if oi < 8:
                                dst = QT[:, oi, tsl]
                            elif oi < 10:
                                dst = KTt[:, oi - 8, tsl]
                            else:
                                dst = VT[:, oi - 10, tsl]
                            # alternate engines so psum slots free faster
                            if o % 2 == 0:
                                nc.scalar.copy(dst, psums[o])
                            else:
                                nc.vector.tensor_copy(dst, psums[o])

            # constants for RoPE/attention — loaded once projections are
            # underway so they don't delay the first weight tiles
            nc.scalar.dma_start(cosT, cos_d.ap())
            nc.scalar.dma_start(sinT, sin_d.ap())
            nc.scalar.dma_start(prot, prot_d.ap())
            nc.scalar.dma_start(cmask, cmask_d.ap())

            # wo load after the x^T/weight stream pools are gone, so it
            # overlaps RoPE + attention without blowing SBUF
            wo_pool = tc.alloc_tile_pool(name="wop", bufs=1)
            wo_t = wo_pool.tile([128, QH, C], BF16)
            nc.sync.dma_start(wo_t, wo_d.ap().rearrange("(h p) n -> p h n", p=128))

            # ---------------- attention (with fused RoPE) ------------------
            # rot = P_rot @ q (sign baked into P_rot), q' = q*cos + rot*sin
            # S^T[k,q] = K @ Q^T; P^T = exp(S^T + mask); O = P @ [V|1]
            with tc.tile_pool(name="pst", bufs=4, space="PSUM") as stp, \
                 tc.tile_pool(name="po", bufs=4, space="PSUM") as pop, \
                 tc.tile_pool(name="pt", bufs=6) as ptp, \
                 tc.tile_pool(name="ob", bufs=4) as obp, \
                 tc.tile_pool(name="ropes", bufs=3) as rsp, \
                 tc.tile_pool(name="rc", bufs=4) as rcp:

                def rope(src):
                    for rqc in range(QC):
                        rsl = slice(rqc * 512, (rqc + 1) * 512)
                        ps = stp.tile([128, 512], F32, name=f"rot{rqc}", tag="st")
                        nc.tensor.matmul(ps, prot, src[:, rsl], start=True,
                                         stop=True)
                        rs = rsp.tile([128, 512], BF16, name=f"rs{rqc}", tag="rs")
                        nc.vector.tensor_tensor(rs, ps, sinT[:, rsl], op=ALU.mult)
                        nc.vector.tensor_tensor(src[:, rsl], src[:, rsl],
                                                cosT[:, rsl], op=ALU.mult)
                        nc.vector.tensor_tensor(src[:, rsl], src[:, rsl], rs,
                                                op=ALU.add)

                def vtrans(kv):
                    for kt in range(KT):
                        pt = stp.tile([128, 128], BF16, name=f"tv{kv}_{kt}",
                                      tag="st")
                        nc.tensor.transpose(
                            pt, VT[:, kv, kt * 128:(kt + 1) * 128], ident)
                        nc.vector.tensor_copy(Vn[:, kv, kt, 0:128], pt)

                rope(KTt[:, 0, :])
                vtrans(0)
                rope(QT[:, 0, :])
                rope(KTt[:, 1, :])
                vtrans(1)

                for h in range(QH):
                    kv = h // 4
                    if h + 1 < QH:
                        rope(QT[:, h + 1, :])
                    for qc in range(QC):
                        tsl = slice(qc * 512, (qc + 1) * 512)
                        po = [pop.tile([128, 129], F32, name=f"po{h}_{qc}_{j}",
                                       tag="po") for j in range(4)]
                        for kt in range(4 * qc + 4):
                            st = stp.tile([128, 512], F32, tag="st")
                            nc.tensor.matmul(
                                st, KTt[:, kv, kt * 128:(kt + 1) * 128],
                                QT[:, h, tsl], start=True, stop=True)
                            d = kt - 4 * qc
                            ptile = ptp.tile([128, 512], BF16)
                            if d >= 0:
                                # columns < d*128 are fully masked (skip);
                                # only the [d*128,(d+1)*128) block straddles
                                # the diagonal and needs the additive mask
                                bsl = slice(d * 128, (d + 1) * 128)
                                vsl = slice(d * 128, 512)
                                nc.vector.tensor_tensor(
                                    st[:, bsl], st[:, bsl], cmask[:, d, bsl],
                                    op=ALU.add)
                                nc.scalar.activation(ptile[:, vsl], st[:, vsl],
                                                     AF.Exp)
                            else:
                                nc.scalar.activation(ptile, st, AF.Exp)
                            for j in range(4):
                                qt = 4 * qc + j
                                if kt <= qt:
                                    nc.tensor.matmul(
                                        po[j], ptile[:, j * 128:(j + 1) * 128],
                                        Vn[:, kv, kt, 0:129],
                                        start=(kt == 0), stop=(kt == qt))
                        for j in range(4):
                            qt = 4 * qc + j
                            rc = rcp.tile([128, 1], F32)
                            nc.vector.reciprocal(rc, po[j][:, 128:129])
                            # store O natural [tok, hd] into OT's block; the
                            # in-place transpose batch below fixes the layout
                            # without stalling the PE mid-attention
                            nc.vector.tensor_scalar_mul(
                                OT[:, h, qt * 128:(qt + 1) * 128],
                                po[j][:, 0:128], rc)

            # ---------------- o_proj partial: O @ wo_slice ----------------
            with tc.tile_pool(name="pout", bufs=6, space="PSUM") as outp, \
                 tc.tile_pool(name="potr", bufs=2, space="PSUM") as otrp, \
                 tc.tile_pool(name="ostg", bufs=6) as stgp:
                # batched in-place transposes: OT blocks [tok,hd] -> [hd,tok]
                for h in range(QH):
                    for qt in range(TT):
                        osl = slice(qt * 128, (qt + 1) * 128)
                        ptr = otrp.tile([128, 128], BF16,
                                        name=f"otr{h}_{qt}", tag="otr")
                        nc.tensor.transpose(ptr, OT[:, h, osl], ident)
                        nc.vector.tensor_copy(OT[:, h, osl], ptr)
                for tt in range(TT):
                    psl = slice(tt * 128, (tt + 1) * 128)
                    for n in range(8):
                        nsl = slice(n * 512, (n + 1) * 512)
                        ps = outp.tile([128, 512], F32)
                        for h in range(QH):
                            nc.tensor.matmul(ps, OT[:, h, psl],
                                             wo_t[:, h, nsl],
                                             start=(h == 0), stop=(h == QH - 1))
                        stg = stgp.tile([128, 512], F32)
                        nc.scalar.copy(stg, ps)
                        nc.sync.dma_start(out_d.ap()[psl, nsl], stg)

            wo_pool.release()

    nc.compile()
    return nc


def _host_prep(x, wq, wk, wv, wo):
    bf = ml_dtypes.bfloat16
    scale = HD ** -0.5

    # RoPE tables, feature-major [128, T]
    inv_freq = 1.0 / (THETA ** (np.arange(0, HD, 2, dtype=np.float32) / HD))
    t = np.arange(T, dtype=np.float32)
    freqs = np.outer(t, inv_freq)                      # [T, 64]
    emb = np.concatenate([freqs, freqs], -1)           # [T, 128]
    cosT = np.ascontiguousarray(np.cos(emb).T).astype(bf)
    sinT = np.ascontiguousarray(np.sin(emb).T).astype(bf)

    # rotate_half as a permutation matrix, pre-transposed for lhsT:
    # rot = P_rot @ q with P_rot[i, i+64] = -1 (i<64), P_rot[i, i-64] = +1.
    protT = np.zeros((128, 128), np.float32)
    for i in range(64):
        protT[i + 64, i] = -1.0
        protT[i, i + 64] = 1.0
    protT = protT.astype(bf)

    ident = np.eye(128, dtype=np.float32).astype(bf)

    # additive causal masks for the 4 diagonal [128k, 512q] tiles
    # valid iff q_local >= d*128 + k_local
    kl = np.arange(128)[:, None]
    ql = np.arange(512)[None, :]
    cmask = np.stack(
        [np.where(ql >= d * 128 + kl, 0.0, -1e9).astype(np.float32)
         for d in range(4)], axis=1)                   # [128, 4, 512]
    cmask = np.ascontiguousarray(cmask)

    xT = []
    for b in range(B):
        xT.append(np.ascontiguousarray(x[b].astype(bf).T))

    wqkv, wob = [], []
    for g in range(4):
        q_s = (wq[:, g * 1024:(g + 1) * 1024] * scale).astype(bf)
        k_s = wk[:, g * 256:(g + 1) * 256].astype(bf)
        v_s = wv[:, g * 256:(g + 1) * 256].astype(bf)
        wall = np.concatenate([q_s, k_s, v_s], axis=1)       # [C, 1536]
        wall = wall.reshape(CCH, 128, 2, 768).transpose(0, 2, 1, 3)
        wqkv.append(np.ascontiguousarray(wall))              # [32, 2, 128, 768]
        wob.append(np.ascontiguousarray(
            wo[g * 1024:(g + 1) * 1024, :].astype(bf)))      # [1024, C]

    in_maps = []
    for core in range(NCORES):
        b, g = core // 4, core % 4
        in_maps.append({
            "xT": xT[b], "wqkv": wqkv[g], "wo": wob[g],
            "cosT": cosT, "sinT": sinT, "protT": protT,
            "ident": ident, "cmask": cmask,
        })
    return in_maps


def kernel(x, wq, wk, wv, wo, _trace=False, _tmpdir=None):
    if "nc" not in _CACHE:
        _CACHE["nc"] = _build_nc()
    nc = _CACHE["nc"]

    in_maps = _host_prep(x, wq, wk, wv, wo)
    res = run_bass_kernel_spmd(nc, in_maps, core_ids=list(range(NCORES)),
                               trace=_trace, tmpdir=_tmpdir)
    _CACHE["last_results"] = res

    out = np.zeros((B, T, C), np.float32)
    for core in range(NCORES):
        out[core // 4] += res.results[core]["out"]
    return out



# revision 6
# speedup vs baseline: 1.2610x; 1.0186x over previous
"""GQA attention kernel for 8 TRN2 NeuronCores.

Problem: B=2, T=2048, C=4096, NH=32 q-heads, NKV=8 kv-heads, HD=128,
RoPE (theta=1e4), causal, f32 I/O.

Sharding: core = (batch b, kv-head-group g): b = core//4, g = core%4.
Each core owns batch b and kv heads {2g, 2g+1} (= q heads 8g..8g+7):
  - projects x[b] against its wq/wk/wv column slices (bf16 compute),
  - runs causal attention for its 8 q heads,
  - computes the partial o_proj x its wo row slice -> [T, C] f32.
Host sums the 4 partials per batch.

On-chip layout is feature-major ("X^T"): activations live as
[feature=partition, token=free] so every matmul contracts along
partitions. x is pre-transposed/bf16-cast on host; RoPE's rotate_half
is a 128x128 permutation matmul on the PE.

Attention uses a "flipped" PV: the P^T tiles produced by the score
matmuls stream through the PE against a stationary V-natural tile,
accumulating O^T (feature-major) directly in PSUM -- 512 columns per
weight load (vs 129 natural, which is LDWEIGHTS-bound) and no
post-attention O transposes. The softmax denominator comes from a
[128,128]-ones matmul over a DVE-accumulated column-sum tile (plus the
last P^T tile directly), already broadcast over partitions; a fast
approximate reciprocal (custom DVE op) and one multiply normalize
O^T on its way out of PSUM. The causal mask is a {0,1} multiply on
the otherwise-idle GpSimd engine, which also carries two of the three
RoPE elementwise ops.
"""

import sys

sys.path.insert(0, "/opt/trn_rl_repo")

import numpy as np
import ml_dtypes

import concourse.bass as bass
import concourse.bacc as bacc
import concourse.mybir as mybir
import concourse.tile as tile
from concourse.bass_utils import run_bass_kernel_spmd

BF16 = mybir.dt.bfloat16
F32 = mybir.dt.float32
AF = mybir.ActivationFunctionType
ALU = mybir.AluOpType

B, T, C = 2, 2048, 4096
NH, NKV, HD = 32, 8, 128
THETA = 10000.0
NCORES = 8

QH = 8          # q heads per core
KV = 2          # kv heads per core
OUTS = 12       # projection output tiles: 8 q + 2 k + 2 v
QC = 4          # token chunks of 512
KT = 16         # k tiles of 128
TT = 16         # token tiles of 128
CCH = 32        # contraction chunks of 128 over C

_CACHE = {}


def _build_nc():
    nc = bacc.Bacc("TRN2", target_bir_lowering=False, debug=False,
                   enable_asserts=False, num_devices=NCORES)

    xT_d = nc.dram_tensor("xT", [C, T], BF16, kind="ExternalInput")
    wqkv_d = nc.dram_tensor("wqkv", [CCH, 2, 128, 768], BF16, kind="ExternalInput")
    wo_d = nc.dram_tensor("wo", [QH * HD, C], BF16, kind="ExternalInput")
    cos_d = nc.dram_tensor("cosT", [128, T], BF16, kind="ExternalInput")
    sin_d = nc.dram_tensor("sinT", [128, T], BF16, kind="ExternalInput")
    prot_d = nc.dram_tensor("protT", [128, 128], BF16, kind="ExternalInput")
    ident_d = nc.dram_tensor("ident", [128, 128], BF16, kind="ExternalInput")
    cmask_d = nc.dram_tensor("cmask01", [128, 4, 128], BF16, kind="ExternalInput")
    out_d = nc.dram_tensor("out", [T, C], F32, kind="ExternalOutput")

    with tile.TileContext(nc) as tc:
        with tc.tile_pool(name="persist", bufs=1) as pp:
            ident = pp.tile([128, 128], BF16)
            nc.sync.dma_start(ident, ident_d.ap())
            cosT = pp.tile([128, T], BF16)
            sinT = pp.tile([128, T], BF16)
            prot = pp.tile([128, 128], BF16)
            cmask = pp.tile([128, 4, 128], BF16)
            ones128 = pp.tile([128, 128], BF16)
            warm = pp.tile([128, 512], BF16)
            nc.vector.memset(ones128, 1.0)
            nc.vector.memset(warm, 0.25)

            # HAM warm-up with REAL matmuls (transposes don't count as
            # PE-busy for the clock gate). No DMA dependency: operand is
            # memset on-chip, so this starts immediately and spans the
            # ~8us until the first x^T block lands, leaving the PE at
            # 2.4 GHz when projections begin. N=512 streams keep the
            # PE-array duty cycle high enough for the HAM window to trip.
            with tc.tile_pool(name="pwarm", bufs=2, space="PSUM") as pwp:
                for w in range(24):
                    wps = pwp.tile([128, 512], F32, name=f"warm{w}", tag="warm")
                    nc.tensor.matmul(wps, warm[:, 0:128], warm,
                                     start=True, stop=True)

            QT = pp.tile([128, QH, T], BF16)
            KTt = pp.tile([128, KV, T], BF16)
            VT = pp.tile([128, KV, T], BF16)
            OT = pp.tile([128, QH, T], BF16)
            Vn = pp.tile([128, KV, KT, 128], BF16)

            # ---------------- projections: Q^T/K^T/V^T = W^T @ x^T ----------
            with tc.tile_pool(name="xt", bufs=2) as xtp, \
                 tc.tile_pool(name="wt", bufs=6) as wtp, \
                 tc.tile_pool(name="pproj", bufs=7, space="PSUM") as ppj, \
                 tc.tile_pool(name="pvt", bufs=1, space="PSUM") as pvt:
                xview = xT_d.ap().rearrange("(c p) t -> p c t", p=128)
                for qc in range(QC):
                    tsl = slice(qc * 512, (qc + 1) * 512)
                    xt = xtp.tile([128, CCH, 512], BF16)
                    # split the load (early c-chunks land first) and use the
                    # scalar HWDGE queue so weights stream in parallel on sync
                    for piece in range(4):
                        csl = slice(piece * 8, (piece + 1) * 8)
                        nc.scalar.dma_start(xt[:, csl, :], xview[:, csl, tsl])
                    for grp in range(2):
                        psums = [ppj.tile([128, 512], F32, name=f"pj{qc}_{grp}_{o}",
                                          tag="pj") for o in range(6)]
                        for c in range(CCH):
                            wt = wtp.tile([128, 768], BF16)
                            nc.sync.dma_start(wt, wqkv_d.ap()[c, grp])
                            for o in range(6):
                                nc.tensor.matmul(
                                    psums[o], wt[:, o * 128:(o + 1) * 128],
                                    xt[:, c, :], start=(c == 0), stop=(c == CCH - 1))
                        for o in range(6):
                            oi = grp * 6 + o
                            if oi < 8:
                                dst = QT[:, oi, tsl]
                            elif oi < 10:
                                dst = KTt[:, oi - 8, tsl]
                            else:
                                dst = VT[:, oi - 10, tsl]
                            # alternate engines so psum slots free faster
                            if o % 2 == 0:
                                nc.scalar.copy(dst, psums[o])
                            else:
                                nc.vector.tensor_copy(dst, psums[o])
                    # V-natural transposes for this token chunk, inside the
                    # proj stream (PE is HAM-warm here; the attention
                    # prologue stays short)
                    for kv_ in range(KV):
                        for kt_ in range(4 * qc, 4 * qc + 4):
                            pt_ = pvt.tile([128, 128], BF16,
                                           name=f"tv{kv_}_{kt_}", tag="vt")
                            nc.tensor.transpose(
                                pt_, VT[:, kv_, kt_ * 128:(kt_ + 1) * 128],
                                ident)
                            nc.scalar.copy(Vn[:, kv_, kt_, :], pt_)

            # constants for RoPE/attention — loaded once projections are
            # underway so they don't delay the first weight tiles
            nc.scalar.dma_start(cosT, cos_d.ap())
            nc.scalar.dma_start(sinT, sin_d.ap())
            nc.scalar.dma_start(prot, prot_d.ap())
            nc.scalar.dma_start(cmask, cmask_d.ap())

            # wo load after the x^T/weight stream pools are gone, so it
            # overlaps RoPE + attention without blowing SBUF
            wo_pool = tc.alloc_tile_pool(name="wop", bufs=1)
            wo_t = wo_pool.tile([128, QH, C], BF16)
            nc.sync.dma_start(wo_t, wo_d.ap().rearrange("(h p) n -> p h n", p=128))

            # ---------------- attention (with fused RoPE) ------------------
            # rot = P_rot @ q (sign baked into P_rot), q' = q*cos + rot*sin
            # S^T[k,q] = K @ Q^T; P^T = exp(S^T) * {0,1}-mask (gpsimd)
            # O^T[hd,q] = sum_kt V_nat[kt]^T @ P^T[kt]   (flipped PV)
            # denB[*,q] = ones128 @ (sum_kt P^T[kt]); O^T *= approx(1/denB)
            with tc.tile_pool(name="pst", bufs=2, space="PSUM") as pst, \
                 tc.tile_pool(name="pot", bufs=2, space="PSUM") as pot, \
                 tc.tile_pool(name="pdb", bufs=2, space="PSUM") as pdb, \
                 tc.tile_pool(name="pt", bufs=3) as ptp, \
                 tc.tile_pool(name="acc", bufs=2) as accp, \
                 tc.tile_pool(name="rcd", bufs=2) as rcdp, \
                 tc.tile_pool(name="ropes", bufs=3) as rsp:

                def rope_chunk(src, rqc, on_gpsimd):
                    rsl = slice(rqc * 512, (rqc + 1) * 512)
                    ps = pdb.tile([128, 512], F32, name=f"rot{rqc}", tag="db")
                    nc.tensor.matmul(ps, prot, src[:, rsl], start=True,
                                     stop=True)
                    rs = rsp.tile([128, 512], BF16, name=f"rs{rqc}", tag="rs")
                    nc.vector.tensor_tensor(rs, ps, sinT[:, rsl], op=ALU.mult)
                    nc.vector.tensor_tensor(src[:, rsl], src[:, rsl],
                                            cosT[:, rsl], op=ALU.mult)
                    if on_gpsimd:
                        # only the final add goes to gpsimd: short op, so
                        # the latency-critical diag masks never queue long
                        nc.gpsimd.tensor_tensor(src[:, rsl], src[:, rsl], rs,
                                                op=ALU.add)
                    else:
                        nc.vector.tensor_tensor(src[:, rsl], src[:, rsl], rs,
                                                op=ALU.add)

                # prologue ropes all-DVE (gpsimd chains here would leave the
                # PE idle long enough to re-throttle HAM); c0 tiles first so
                # head 0 can start as soon as possible
                rope_chunk(KTt[:, 0, :], 0, False)
                rope_chunk(QT[:, 0, :], 0, False)
                for rqc in range(1, QC):
                    rope_chunk(KTt[:, 0, :], rqc, False)
                    rope_chunk(QT[:, 0, :], rqc, False)
                for rqc in range(QC):
                    rope_chunk(KTt[:, 1, :], rqc, False)

                pending = [None]

                def flush_pending():
                    if pending[0] is None:
                        return
                    p_acc, p_lastpt, p_ot, p_h, p_tsl = pending[0]
                    pending[0] = None
                    denB = pdb.tile([128, 512], F32, tag="db")
                    nc.tensor.matmul(denB, ones128, p_acc,
                                     start=True, stop=False)
                    nc.tensor.matmul(denB[:, 384:512], ones128,
                                     p_lastpt[:, 384:512],
                                     start=False, stop=True)
                    rcdB = rcdp.tile([128, 512], F32)
                    nc.vector.reciprocal_approx_fast(rcdB, denB)
                    nc.vector.tensor_tensor(OT[:, p_h, p_tsl], p_ot, rcdB,
                                            op=ALU.mult)

                for h in range(QH):
                    kv = h // 4
                    for qc in range(QC):
                        tsl = slice(qc * 512, (qc + 1) * 512)
                        nk = 4 * qc + 4       # k tiles this chunk (even)
                        last_kt = nk - 1
                        otps = pot.tile([128, 512], F32, name=f"ot{h}_{qc}",
                                        tag="ot")
                        acc = accp.tile([128, 512], BF16)

                        # stage 1 of the pair pipeline: score matmuls + exp
                        # (+ gpsimd {0,1} mask for diagonal tiles)
                        def scores(kp, h=h, kv=kv, qc=qc):
                            kt0 = 2 * kp
                            st_pair = pst.tile([128, 2, 512], F32, tag="st")
                            pt_pair = ptp.tile([128, 2, 512], BF16)
                            if kt0 + 1 < 4 * qc:          # full pair
                                for half in (0, 1):
                                    kt = kt0 + half
                                    nc.tensor.matmul(
                                        st_pair[:, half, :],
                                        KTt[:, kv, kt * 128:(kt + 1) * 128],
                                        QT[:, h, qc * 512:(qc + 1) * 512],
                                        start=True, stop=True)
                                # one ACTIVATE covers the whole pair
                                nc.scalar.activation(pt_pair, st_pair, AF.Exp)
                            else:                          # diagonal pair
                                for half in (0, 1):
                                    kt = kt0 + half
                                    d = kt - 4 * qc
                                    vsl = slice(d * 128, 512)
                                    nc.tensor.matmul(
                                        st_pair[:, half, vsl],
                                        KTt[:, kv, kt * 128:(kt + 1) * 128],
                                        QT[:, h,
                                           qc * 512 + d * 128:(qc + 1) * 512],
                                        start=True, stop=True)
                                    nc.scalar.activation(
                                        pt_pair[:, half, vsl],
                                        st_pair[:, half, vsl], AF.Exp)
                                    # zero the above-diagonal part of the
                                    # block straddling the diagonal
                                    nc.gpsimd.tensor_tensor(
                                        pt_pair[:, half,
                                                d * 128:(d + 1) * 128],
                                        pt_pair[:, half,
                                                d * 128:(d + 1) * 128],
                                        cmask[:, d, :], op=ALU.mult)
                            return pt_pair

                        # stage 2: PV accumulation + denominator column-sums
                        def consume(kp, pt_pair, h=h, kv=kv, qc=qc):
                            for half in (0, 1):
                                kt = 2 * kp + half
                                d = kt - 4 * qc
                                vsl = slice(max(d, 0) * 128, 512)
                                ptv = pt_pair[:, half, :]
                                nc.tensor.matmul(
                                    otps[:, vsl], Vn[:, kv, kt, :],
                                    ptv[:, vsl],
                                    start=(kt == 0), stop=(kt == last_kt))
                                if kt == 0 and qc == 0:
                                    nc.vector.tensor_copy(acc, ptv)
                                elif kt == 1 and qc > 0:
                                    nc.vector.tensor_tensor(
                                        acc, pt_pair[:, 0, :],
                                        pt_pair[:, 1, :], op=ALU.add)
                                elif (kt > 1 or qc == 0) and kt < last_kt:
                                    nc.vector.tensor_tensor(
                                        acc[:, vsl], acc[:, vsl], ptv[:, vsl],
                                        op=ALU.add)
                            return pt_pair[:, 1, :]

                        npairs = nk // 2
                        pts = [scores(0)]
                        last_pt = None
                        for kp in range(npairs):
                            if kp + 1 < npairs:
                                pts.append(scores(kp + 1))
                            if kp == 0:
                                # previous chunk's denominator + normalize,
                                # now that its DVE column-sums are long done
                                flush_pending()
                            last_pt = consume(kp, pts[kp])
                        pending[0] = (acc, last_pt, otps, h, tsl)
                        # next head's RoPE, one 512-chunk per chunk-tail,
                        # shifted one chunk early so the gpsimd add never
                        # delays the next chunk's diag masks
                        if qc >= 1 and h + 1 < QH:
                            rope_chunk(QT[:, h + 1, :], qc - 1, True)
                        elif qc == 0 and h >= 1:
                            rope_chunk(QT[:, h, :], 3, True)
                flush_pending()

            # ---------------- o_proj partial: O @ wo_slice ----------------
            with tc.tile_pool(name="pout", bufs=6, space="PSUM") as outp, \
                 tc.tile_pool(name="ostg", bufs=6) as stgp:
                for tt in range(TT):
                    psl = slice(tt * 128, (tt + 1) * 128)
                    for n in range(8):
                        nsl = slice(n * 512, (n + 1) * 512)
                        ps = outp.tile([128, 512], F32)
                        for h in range(QH):
                            nc.tensor.matmul(ps, OT[:, h, psl],
                                             wo_t[:, h, nsl],
                                             start=(h == 0), stop=(h == QH - 1))
                        stg = stgp.tile([128, 512], F32)
                        nc.scalar.copy(stg, ps)
                        nc.sync.dma_start(out_d.ap()[psl, nsl], stg)

            wo_pool.release()

    nc.compile()
    return nc


def _host_prep(x, wq, wk, wv, wo):
    bf = ml_dtypes.bfloat16
    scale = HD ** -0.5

    # RoPE tables, feature-major [128, T]
    inv_freq = 1.0 / (THETA ** (np.arange(0, HD, 2, dtype=np.float32) / HD))
    t = np.arange(T, dtype=np.float32)
    freqs = np.outer(t, inv_freq)                      # [T, 64]
    emb = np.concatenate([freqs, freqs], -1)           # [T, 128]
    cosT = np.ascontiguousarray(np.cos(emb).T).astype(bf)
    sinT = np.ascontiguousarray(np.sin(emb).T).astype(bf)

    # rotate_half as a permutation matrix, pre-transposed for lhsT:
    # rot = P_rot @ q with P_rot[i, i+64] = -1 (i<64), P_rot[i, i-64] = +1.
    protT = np.zeros((128, 128), np.float32)
    for i in range(64):
        protT[i + 64, i] = -1.0
        protT[i, i + 64] = 1.0
    protT = protT.astype(bf)

    ident = np.eye(128, dtype=np.float32).astype(bf)

    # {0,1} causal masks for the diagonal [128k, 128q] blocks of the 4
    # diagonal tiles of each 512-q chunk: valid iff q_local >= k_local
    # (block-local indices coincide for every d)
    kl = np.arange(128)[:, None]
    ql = np.arange(128)[None, :]
    blk = (ql >= kl).astype(np.float32)
    cmask01 = np.stack([blk] * 4, axis=1).astype(bf)   # [128, 4, 128]
    cmask01 = np.ascontiguousarray(cmask01)

    xT = []
    for b in range(B):
        xT.append(np.ascontiguousarray(x[b].astype(bf).T))

    wqkv, wob = [], []
    for g in range(4):
        q_s = (wq[:, g * 1024:(g + 1) * 1024] * scale).astype(bf)
        k_s = wk[:, g * 256:(g + 1) * 256].astype(bf)
        v_s = wv[:, g * 256:(g + 1) * 256].astype(bf)
        wall = np.concatenate([q_s, k_s, v_s], axis=1)       # [C, 1536]
        wall = wall.reshape(CCH, 128, 2, 768).transpose(0, 2, 1, 3)
        wqkv.append(np.ascontiguousarray(wall))              # [32, 2, 128, 768]
        wob.append(np.ascontiguousarray(
            wo[g * 1024:(g + 1) * 1024, :].astype(bf)))      # [1024, C]

    in_maps = []
    for core in range(NCORES):
        b, g = core // 4, core % 4
        in_maps.append({
            "xT": xT[b], "wqkv": wqkv[g], "wo": wob[g],
            "cosT": cosT, "sinT": sinT, "protT": protT,
            "ident": ident, "cmask01": cmask01,
        })
    return in_maps


def kernel(x, wq, wk, wv, wo, _trace=False, _tmpdir=None):
    if "nc" not in _CACHE:
        _CACHE["nc"] = _build_nc()
    nc = _CACHE["nc"]

    in_maps = _host_prep(x, wq, wk, wv, wo)
    res = run_bass_kernel_spmd(nc, in_maps, core_ids=list(range(NCORES)),
                               trace=_trace, tmpdir=_tmpdir)
    _CACHE["last_results"] = res

    out = np.zeros((B, T, C), np.float32)
    for core in range(NCORES):
        out[core // 4] += res.results[core]["out"]
    return out


# revision 8
# speedup vs baseline: 1.2684x; 1.0058x over previous
"""GQA attention kernel for 8 TRN2 NeuronCores.

Problem: B=2, T=2048, C=4096, NH=32 q-heads, NKV=8 kv-heads, HD=128,
RoPE (theta=1e4), causal, f32 I/O.

Sharding: core = (batch b, kv-head-group g): b = core//4, g = core%4.
Each core owns batch b and kv heads {2g, 2g+1} (= q heads 8g..8g+7):
  - projects x[b] against its wq/wk/wv column slices (bf16 compute),
  - runs causal attention for its 8 q heads,
  - computes the partial o_proj x its wo row slice -> [T, C] f32.
Host sums the 4 partials per batch.

On-chip layout is feature-major ("X^T"): activations live as
[feature=partition, token=free] so every matmul contracts along
partitions. x is pre-transposed/bf16-cast on host; RoPE's rotate_half
is a 128x128 permutation matmul on the PE.

Attention uses a "flipped" PV: the P^T tiles produced by the score
matmuls stream through the PE against a stationary V-natural tile,
accumulating O^T (feature-major) directly in PSUM -- 512 columns per
weight load (vs 129 natural, which is LDWEIGHTS-bound) and no
post-attention O transposes. The softmax denominator comes from a
[128,128]-ones matmul over a DVE-accumulated column-sum tile (plus the
last P^T tile directly), already broadcast over partitions; a fast
approximate reciprocal (custom DVE op) and one multiply normalize
O^T on its way out of PSUM. The causal mask is a {0,1} multiply on
the otherwise-idle GpSimd engine, which also carries two of the three
RoPE elementwise ops.
"""

import sys

sys.path.insert(0, "/opt/trn_rl_repo")

import numpy as np
import ml_dtypes

import concourse.bass as bass
import concourse.bacc as bacc
import concourse.mybir as mybir
import concourse.tile as tile
from concourse.bass_utils import run_bass_kernel_spmd

BF16 = mybir.dt.bfloat16
F32 = mybir.dt.float32
AF = mybir.ActivationFunctionType
ALU = mybir.AluOpType

B, T, C = 2, 2048, 4096
NH, NKV, HD = 32, 8, 128
THETA = 10000.0
NCORES = 8

QH = 8          # q heads per core
KV = 2          # kv heads per core
OUTS = 12       # projection output tiles: 8 q + 2 k + 2 v
QC = 4          # token chunks of 512
KT = 16         # k tiles of 128
TT = 16         # token tiles of 128
CCH = 32        # contraction chunks of 128 over C

_CACHE = {}


def _build_nc():
    nc = bacc.Bacc("TRN2", target_bir_lowering=False, debug=False,
                   enable_asserts=False, num_devices=NCORES)

    xT_d = nc.dram_tensor("xT", [C, T], BF16, kind="ExternalInput")
    wqkv_d = nc.dram_tensor("wqkv", [CCH, 2, 128, 768], BF16, kind="ExternalInput")
    wo_d = nc.dram_tensor("wo", [QH * HD, C], BF16, kind="ExternalInput")
    cos_d = nc.dram_tensor("cosT", [128, T], BF16, kind="ExternalInput")
    sin_d = nc.dram_tensor("sinT", [128, T], BF16, kind="ExternalInput")
    prot_d = nc.dram_tensor("protT", [128, 128], BF16, kind="ExternalInput")
    ident_d = nc.dram_tensor("ident", [128, 128], BF16, kind="ExternalInput")
    cmask_d = nc.dram_tensor("cmask01", [128, 4, 128], BF16, kind="ExternalInput")
    out_d = nc.dram_tensor("out", [T, C], F32, kind="ExternalOutput")

    with tile.TileContext(nc) as tc:
        with tc.tile_pool(name="persist", bufs=1) as pp:
            ident = pp.tile([128, 128], BF16)
            nc.sync.dma_start(ident, ident_d.ap())
            cosT = pp.tile([128, T], BF16)
            sinT = pp.tile([128, T], BF16)
            prot = pp.tile([128, 128], BF16)
            cmask = pp.tile([128, 4, 128], BF16)
            ones128 = pp.tile([128, 128], BF16)
            warm = pp.tile([128, 512], BF16)
            nc.vector.memset(ones128, 1.0)
            nc.vector.memset(warm, 0.25)

            # HAM warm-up with REAL matmuls (transposes don't count as
            # PE-busy for the clock gate). No DMA dependency: operand is
            # memset on-chip, so this starts immediately and spans the
            # ~8us until the first x^T block lands, leaving the PE at
            # 2.4 GHz when projections begin. N=512 streams keep the
            # PE-array duty cycle high enough for the HAM window to trip.
            with tc.tile_pool(name="pwarm", bufs=2, space="PSUM") as pwp:
                for w in range(24):
                    wps = pwp.tile([128, 512], F32, name=f"warm{w}", tag="warm")
                    nc.tensor.matmul(wps, warm[:, 0:128], warm,
                                     start=True, stop=True)

            QT = pp.tile([128, QH, T], BF16)
            KTt = pp.tile([128, KV, T], BF16)
            VT = pp.tile([128, KV, T], BF16)
            OT = pp.tile([128, QH, T], BF16)
            Vn = pp.tile([128, KV, KT, 128], BF16)

            def rope_chunk_in(pool, rsp_pool, src, rqc, add_engine):
                rsl = slice(rqc * 512, (rqc + 1) * 512)
                ps = pool.tile([128, 512], F32, name=f"rotp{rqc}", tag="vt")
                nc.tensor.matmul(ps, prot, src[:, rsl], start=True, stop=True)
                rs = rsp_pool.tile([128, 512], BF16, name=f"rsp{rqc}", tag="rs")
                nc.vector.tensor_tensor(rs, ps, sinT[:, rsl], op=ALU.mult)
                nc.vector.tensor_tensor(src[:, rsl], src[:, rsl],
                                        cosT[:, rsl], op=ALU.mult)
                add_engine.tensor_tensor(src[:, rsl], src[:, rsl], rs,
                                         op=ALU.add)

            # ---------------- projections: Q^T/K^T/V^T = W^T @ x^T ----------
            with tc.tile_pool(name="xt", bufs=2) as xtp, \
                 tc.tile_pool(name="wt", bufs=6) as wtp, \
                 tc.tile_pool(name="pproj", bufs=7, space="PSUM") as ppj, \
                 tc.tile_pool(name="pvt", bufs=1, space="PSUM") as pvt, \
                 tc.tile_pool(name="prs", bufs=2) as prs:
                xview = xT_d.ap().rearrange("(c p) t -> p c t", p=128)
                for qc in range(QC):
                    tsl = slice(qc * 512, (qc + 1) * 512)
                    xt = xtp.tile([128, CCH, 512], BF16)
                    # split the load (early c-chunks land first) and use the
                    # scalar HWDGE queue so weights stream in parallel on sync
                    for piece in range(4):
                        csl = slice(piece * 8, (piece + 1) * 8)
                        nc.scalar.dma_start(xt[:, csl, :], xview[:, csl, tsl])
                    if qc == 0:
                        # RoPE/mask tables: after the first x^T block (so
                        # they don't delay the first matmuls) but early
                        # enough for the in-proj rope of chunk 0
                        nc.scalar.dma_start(cosT, cos_d.ap())
                        nc.scalar.dma_start(sinT, sin_d.ap())
                        nc.scalar.dma_start(prot, prot_d.ap())
                        nc.scalar.dma_start(cmask, cmask_d.ap())
                    for grp in range(2):
                        psums = [ppj.tile([128, 512], F32, name=f"pj{qc}_{grp}_{o}",
                                          tag="pj") for o in range(6)]
                        for c in range(CCH):
                            wt = wtp.tile([128, 768], BF16)
                            nc.sync.dma_start(wt, wqkv_d.ap()[c, grp])
                            for o in range(6):
                                nc.tensor.matmul(
                                    psums[o], wt[:, o * 128:(o + 1) * 128],
                                    xt[:, c, :], start=(c == 0), stop=(c == CCH - 1))
                        for o in range(6):
                            oi = grp * 6 + o
                            if oi < 8:
                                dst = QT[:, oi, tsl]
                            elif oi < 10:
                                dst = KTt[:, oi - 8, tsl]
                            else:
                                dst = VT[:, oi - 10, tsl]
                            # alternate engines so psum slots free faster
                            if o % 2 == 0:
                                nc.scalar.copy(dst, psums[o])
                            else:
                                nc.vector.tensor_copy(dst, psums[o])
                    # V-natural transposes for this token chunk, inside the
                    # proj stream (PE is HAM-warm here; the attention
                    # prologue stays short)
                    for kv_ in range(KV):
                        for kt_ in range(4 * qc, 4 * qc + 4):
                            pt_ = pvt.tile([128, 128], BF16,
                                           name=f"tv{kv_}_{kt_}", tag="vt")
                            nc.tensor.transpose(
                                pt_, VT[:, kv_, kt_ * 128:(kt_ + 1) * 128],
                                ident)
                            nc.scalar.copy(Vn[:, kv_, kt_, :], pt_)
                    # RoPE for K heads + first Q head rides inside the proj
                    # stream too: head 0 can start the moment proj ends
                    rope_chunk_in(pvt, prs, KTt[:, 0, :], qc, nc.vector)
                    rope_chunk_in(pvt, prs, QT[:, 0, :], qc, nc.vector)
                    rope_chunk_in(pvt, prs, KTt[:, 1, :], qc, nc.vector)

            # wo load after the x^T/weight stream pools are gone, so it
            # overlaps RoPE + attention without blowing SBUF
            wo_pool = tc.alloc_tile_pool(name="wop", bufs=1)
            wo_t = wo_pool.tile([128, QH, C], BF16)
            nc.sync.dma_start(wo_t, wo_d.ap().rearrange("(h p) n -> p h n", p=128))

            # ---------------- attention (with fused RoPE) ------------------
            # rot = P_rot @ q (sign baked into P_rot), q' = q*cos + rot*sin
            # S^T[k,q] = K @ Q^T; P^T = exp(S^T) * {0,1}-mask (gpsimd)
            # O^T[hd,q] = sum_kt V_nat[kt]^T @ P^T[kt]   (flipped PV)
            # denB[*,q] = ones128 @ (sum_kt P^T[kt]); O^T *= approx(1/denB)
            with tc.tile_pool(name="pst", bufs=2, space="PSUM") as pst, \
                 tc.tile_pool(name="pot", bufs=2, space="PSUM") as pot, \
                 tc.tile_pool(name="pdb", bufs=2, space="PSUM") as pdb, \
                 tc.tile_pool(name="pt", bufs=3) as ptp, \
                 tc.tile_pool(name="acc", bufs=2) as accp, \
                 tc.tile_pool(name="rcd", bufs=2) as rcdp, \
                 tc.tile_pool(name="ropes", bufs=3) as rsp:

                def rope_chunk(src, rqc, on_gpsimd):
                    rsl = slice(rqc * 512, (rqc + 1) * 512)
                    ps = pdb.tile([128, 512], F32, name=f"rot{rqc}", tag="db")
                    nc.tensor.matmul(ps, prot, src[:, rsl], start=True,
                                     stop=True)
                    rs = rsp.tile([128, 512], BF16, name=f"rs{rqc}", tag="rs")
                    nc.vector.tensor_tensor(rs, ps, sinT[:, rsl], op=ALU.mult)
                    nc.vector.tensor_tensor(src[:, rsl], src[:, rsl],
                                            cosT[:, rsl], op=ALU.mult)
                    if on_gpsimd:
                        # only the final add goes to gpsimd: short op, so
                        # the latency-critical diag masks never queue long
                        nc.gpsimd.tensor_tensor(src[:, rsl], src[:, rsl], rs,
                                                op=ALU.add)
                    else:
                        nc.vector.tensor_tensor(src[:, rsl], src[:, rsl], rs,
                                                op=ALU.add)


                pending = [None]

                def flush_pending():
                    if pending[0] is None:
                        return
                    p_acc, p_lastpt, p_ot, p_h, p_tsl = pending[0]
                    pending[0] = None
                    denB = pdb.tile([128, 512], F32, tag="db")
                    nc.tensor.matmul(denB, ones128, p_acc,
                                     start=True, stop=False)
                    nc.tensor.matmul(denB[:, 384:512], ones128,
                                     p_lastpt[:, 384:512],
                                     start=False, stop=True)
                    rcdB = rcdp.tile([128, 512], F32)
                    nc.vector.reciprocal_approx_fast(rcdB, denB)
                    nc.vector.tensor_tensor(OT[:, p_h, p_tsl], p_ot, rcdB,
                                            op=ALU.mult)

                for h in range(QH):
                    kv = h // 4
                    for qc in range(QC):
                        tsl = slice(qc * 512, (qc + 1) * 512)
                        nk = 4 * qc + 4       # k tiles this chunk (even)
                        last_kt = nk - 1
                        otps = pot.tile([128, 512], F32, name=f"ot{h}_{qc}",
                                        tag="ot")
                        acc = accp.tile([128, 512], BF16)

                        # stage 1 of the pair pipeline: score matmuls + exp
                        # (+ gpsimd {0,1} mask for diagonal tiles)
                        def scores(kp, h=h, kv=kv, qc=qc):
                            kt0 = 2 * kp
                            st_pair = pst.tile([128, 2, 512], F32, tag="st")
                            pt_pair = ptp.tile([128, 2, 512], BF16)
                            if kt0 + 1 < 4 * qc:          # full pair
                                for half in (0, 1):
                                    kt = kt0 + half
                                    nc.tensor.matmul(
                                        st_pair[:, half, :],
                                        KTt[:, kv, kt * 128:(kt + 1) * 128],
                                        QT[:, h, qc * 512:(qc + 1) * 512],
                                        start=True, stop=True)
                                # one ACTIVATE covers the whole pair
                                nc.scalar.activation(pt_pair, st_pair, AF.Exp)
                            else:                          # diagonal pair
                                d0 = kt0 - 4 * qc
                                for half in (0, 1):
                                    kt = kt0 + half
                                    d = kt - 4 * qc
                                    vsl = slice(d * 128, 512)
                                    nc.tensor.matmul(
                                        st_pair[:, half, vsl],
                                        KTt[:, kv, kt * 128:(kt + 1) * 128],
                                        QT[:, h,
                                           qc * 512 + d * 128:(qc + 1) * 512],
                                        start=True, stop=True)
                                    if d0 == 0 and half == 1:
                                        # (d0,d1) pair: one full ACTIVATE is
                                        # cheaper than two partials; the
                                        # never-read junk cols are harmless
                                        nc.scalar.activation(
                                            pt_pair, st_pair, AF.Exp)
                                    elif d0 > 0:
                                        nc.scalar.activation(
                                            pt_pair[:, half, vsl],
                                            st_pair[:, half, vsl], AF.Exp)
                                    # zero the above-diagonal part of the
                                    # block straddling the diagonal
                                    if d0 > 0 or half == 1:
                                        for hh in ((0, 1) if (d0 == 0 and
                                                              half == 1)
                                                   else (half,)):
                                            dd = kt0 + hh - 4 * qc
                                            nc.gpsimd.tensor_tensor(
                                                pt_pair[:, hh,
                                                        dd * 128:(dd + 1) * 128],
                                                pt_pair[:, hh,
                                                        dd * 128:(dd + 1) * 128],
                                                cmask[:, dd, :], op=ALU.mult)
                            return pt_pair

                        # stage 2: PV accumulation + denominator column-sums
                        def consume(kp, pt_pair, h=h, kv=kv, qc=qc):
                            for half in (0, 1):
                                kt = 2 * kp + half
                                d = kt - 4 * qc
                                vsl = slice(max(d, 0) * 128, 512)
                                ptv = pt_pair[:, half, :]
                                nc.tensor.matmul(
                                    otps[:, vsl], Vn[:, kv, kt, :],
                                    ptv[:, vsl],
                                    start=(kt == 0), stop=(kt == last_kt))
                                if kt == 0 and qc == 0:
                                    nc.vector.tensor_copy(acc, ptv)
                                elif kt == 1 and qc > 0:
                                    nc.vector.tensor_tensor(
                                        acc, pt_pair[:, 0, :],
                                        pt_pair[:, 1, :], op=ALU.add)
                                elif (kt > 1 or qc == 0) and kt < last_kt:
                                    nc.vector.tensor_tensor(
                                        acc[:, vsl], acc[:, vsl], ptv[:, vsl],
                                        op=ALU.add)
                            return pt_pair[:, 1, :]

                        npairs = nk // 2
                        pts = [scores(0)]
                        last_pt = None
                        for kp in range(npairs):
                            if kp + 1 < npairs:
                                pts.append(scores(kp + 1))
                            if kp == 0:
                                # previous chunk's denominator + normalize,
                                # now that its DVE column-sums are long done
                                flush_pending()
                            last_pt = consume(kp, pts[kp])
                        pending[0] = (acc, last_pt, otps, h, tsl)
                        # next head's RoPE, one 512-chunk per chunk-tail,
                        # shifted one chunk early so the gpsimd add never
                        # delays the next chunk's diag masks
                        if qc >= 1 and h + 1 < QH:
                            rope_chunk(QT[:, h + 1, :], qc - 1, True)
                        elif qc == 0 and h >= 1:
                            rope_chunk(QT[:, h, :], 3, True)
                flush_pending()

            # ---------------- o_proj partial: O @ wo_slice ----------------
            with tc.tile_pool(name="pout", bufs=6, space="PSUM") as outp, \
                 tc.tile_pool(name="ostg", bufs=6) as stgp:
                for tt in range(TT):
                    psl = slice(tt * 128, (tt + 1) * 128)
                    for n in range(8):
                        nsl = slice(n * 512, (n + 1) * 512)
                        ps = outp.tile([128, 512], F32)
                        for h in range(QH):
                            nc.tensor.matmul(ps, OT[:, h, psl],
                                             wo_t[:, h, nsl],
                                             start=(h == 0), stop=(h == QH - 1))
                        stg = stgp.tile([128, 512], F32)
                        nc.scalar.copy(stg, ps)
                        nc.sync.dma_start(out_d.ap()[psl, nsl], stg)

            wo_pool.release()

    nc.compile()
    return nc


def _host_prep(x, wq, wk, wv, wo):
    bf = ml_dtypes.bfloat16
    scale = HD ** -0.5

    # RoPE tables, feature-major [128, T]
    inv_freq = 1.0 / (THETA ** (np.arange(0, HD, 2, dtype=np.float32) / HD))
    t = np.arange(T, dtype=np.float32)
    freqs = np.outer(t, inv_freq)                      # [T, 64]
    emb = np.concatenate([freqs, freqs], -1)           # [T, 128]
    cosT = np.ascontiguousarray(np.cos(emb).T).astype(bf)
    sinT = np.ascontiguousarray(np.sin(emb).T).astype(bf)

    # rotate_half as a permutation matrix, pre-transposed for lhsT:
    # rot = P_rot @ q with P_rot[i, i+64] = -1 (i<64), P_rot[i, i-64] = +1.
    protT = np.zeros((128, 128), np.float32)
    for i in range(64):
        protT[i + 64, i] = -1.0
        protT[i, i + 64] = 1.0
    protT = protT.astype(bf)

    ident = np.eye(128, dtype=np.float32).astype(bf)

    # {0,1} causal masks for the diagonal [128k, 128q] blocks of the 4
    # diagonal tiles of each 512-q chunk: valid iff q_local >= k_local
    # (block-local indices coincide for every d)
    kl = np.arange(128)[:, None]
    ql = np.arange(128)[None, :]
    blk = (ql >= kl).astype(np.float32)
    cmask01 = np.stack([blk] * 4, axis=1).astype(bf)   # [128, 4, 128]
    cmask01 = np.ascontiguousarray(cmask01)

    xT = []
    for b in range(B):
        xT.append(np.ascontiguousarray(x[b].astype(bf).T))

    wqkv, wob = [], []
    for g in range(4):
        q_s = (wq[:, g * 1024:(g + 1) * 1024] * scale).astype(bf)
        k_s = wk[:, g * 256:(g + 1) * 256].astype(bf)
        v_s = wv[:, g * 256:(g + 1) * 256].astype(bf)
        wall = np.concatenate([q_s, k_s, v_s], axis=1)       # [C, 1536]
        wall = wall.reshape(CCH, 128, 2, 768).transpose(0, 2, 1, 3)
        wqkv.append(np.ascontiguousarray(wall))              # [32, 2, 128, 768]
        wob.append(np.ascontiguousarray(
            wo[g * 1024:(g + 1) * 1024, :].astype(bf)))      # [1024, C]

    in_maps = []
    for core in range(NCORES):
        b, g = core // 4, core % 4
        in_maps.append({
            "xT": xT[b], "wqkv": wqkv[g], "wo": wob[g],
            "cosT": cosT, "sinT": sinT, "protT": protT,
            "ident": ident, "cmask01": cmask01,
        })
    return in_maps


def kernel(x, wq, wk, wv, wo, _trace=False, _tmpdir=None):
    if "nc" not in _CACHE:
        _CACHE["nc"] = _build_nc()
    nc = _CACHE["nc"]

    in_maps = _host_prep(x, wq, wk, wv, wo)
    res = run_bass_kernel_spmd(nc, in_maps, core_ids=list(range(NCORES)),
                               trace=_trace, tmpdir=_tmpdir)
    _CACHE["last_results"] = res

    out = np.zeros((B, T, C), np.float32)
    for core in range(NCORES):
        out[core // 4] += res.results[core]["out"]
    return out


# revision 10
# speedup vs baseline: 1.3542x; 1.0676x over previous
"""GQA attention kernel for 8 TRN2 NeuronCores.

Problem: B=2, T=2048, C=4096, NH=32 q-heads, NKV=8 kv-heads, HD=128,
RoPE (theta=1e4), causal, f32 I/O.

Sharding: core = (batch b, kv-head-group g): b = core//4, g = core%4.

v5: the projection chunk loop and the attention "stages" are emitted
interleaved: attention stage qc (all 8 heads' scores/PV/softmax for
q-chunk qc, k<=qc) rides inside proj chunk qc+1's contraction loop, so
the softmax exp (ACT engine, ~180us total) hides under the PE-bound
projection stream instead of serializing after it. Stage 3 runs
post-proj with an extra score-pair pool. Flipped PV + broadcast
denominator as in v4 (see kernel_v4.py docstring).
"""

import sys

sys.path.insert(0, "/opt/trn_rl_repo")

import numpy as np
import ml_dtypes

import concourse.bass as bass
import concourse.bacc as bacc
import concourse.mybir as mybir
import concourse.tile as tile
from concourse.bass_utils import run_bass_kernel_spmd

BF16 = mybir.dt.bfloat16
F32 = mybir.dt.float32
AF = mybir.ActivationFunctionType
ALU = mybir.AluOpType

B, T, C = 2, 2048, 4096
NH, NKV, HD = 32, 8, 128
THETA = 10000.0
NCORES = 8

QH = 8
KV = 2
QC = 4
KT = 16
TT = 16
CCH = 32

_CACHE = {}


def _build_nc():
    nc = bacc.Bacc("TRN2", target_bir_lowering=False, debug=False,
                   enable_asserts=False, num_devices=NCORES)

    xT_d = nc.dram_tensor("xT", [C, T], BF16, kind="ExternalInput")
    wqkv_d = nc.dram_tensor("wqkv", [CCH, 3, 128, 512], BF16, kind="ExternalInput")
    wo_d = nc.dram_tensor("wo", [QH * HD, C], BF16, kind="ExternalInput")
    cos_d = nc.dram_tensor("cosT", [128, T], BF16, kind="ExternalInput")
    sin_d = nc.dram_tensor("sinT", [128, T], BF16, kind="ExternalInput")
    prot_d = nc.dram_tensor("protT", [128, 128], BF16, kind="ExternalInput")
    ident_d = nc.dram_tensor("ident", [128, 128], BF16, kind="ExternalInput")
    cmask_d = nc.dram_tensor("cmask01", [128, 4, 128], BF16, kind="ExternalInput")
    out_d = nc.dram_tensor("out", [T, C], F32, kind="ExternalOutput")

    with tile.TileContext(nc) as tc:
        with tc.tile_pool(name="persist", bufs=1) as pp:
            ident = pp.tile([128, 128], BF16)
            nc.sync.dma_start(ident, ident_d.ap())
            cosT = pp.tile([128, T], BF16)
            sinT = pp.tile([128, T], BF16)
            prot = pp.tile([128, 128], BF16)
            cmask = pp.tile([128, 4, 128], BF16)
            ones128 = pp.tile([128, 128], BF16)
            warm = pp.tile([128, 512], BF16)
            nc.vector.memset(ones128, 1.0)
            nc.vector.memset(warm, 0.25)

            with tc.tile_pool(name="pwarm", bufs=2, space="PSUM") as pwp:
                for w in range(24):
                    wps = pwp.tile([128, 512], F32, name=f"warm{w}", tag="warm")
                    nc.tensor.matmul(wps, warm[:, 0:128], warm,
                                     start=True, stop=True)

            QT = pp.tile([128, QH, T], BF16)
            KTt = pp.tile([128, KV, T], BF16)
            VT = pp.tile([128, KV, T], BF16)
            OT = pp.tile([128, QH, T], BF16)
            Vn = pp.tile([128, KV, KT, 128], BF16)

            # attention pools live for both the merged phase and stage 3
            with tc.tile_pool(name="pst", bufs=1, space="PSUM") as pst, \
                 tc.tile_pool(name="pot", bufs=1, space="PSUM") as pot, \
                 tc.tile_pool(name="pdb", bufs=1, space="PSUM") as pdb, \
                 tc.tile_pool(name="pt", bufs=3) as ptp, \
                 tc.tile_pool(name="acc", bufs=2) as accp, \
                 tc.tile_pool(name="rcd", bufs=2) as rcdp, \
                 tc.tile_pool(name="ropes", bufs=3) as rsp:

                def rope_chunk(src, rqc):
                    rsl = slice(rqc * 512, (rqc + 1) * 512)
                    ps = pdb.tile([128, 512], F32, name=f"rot{rqc}", tag="db")
                    nc.tensor.matmul(ps, prot, src[:, rsl], start=True,
                                     stop=True)
                    rs = rsp.tile([128, 512], BF16, name=f"rs{rqc}", tag="rs")
                    nc.vector.tensor_tensor(rs, ps, sinT[:, rsl], op=ALU.mult)
                    nc.vector.tensor_tensor(src[:, rsl], src[:, rsl],
                                            cosT[:, rsl], op=ALU.mult)
                    nc.vector.tensor_tensor(src[:, rsl], src[:, rsl], rs,
                                            op=ALU.add)

                pending = [None]

                def flush_pending():
                    if pending[0] is None:
                        return
                    p_acc, p_lastpt, p_ot, p_h, p_tsl = pending[0]
                    pending[0] = None
                    denB = pdb.tile([128, 512], F32, tag="db")
                    nc.tensor.matmul(denB, ones128, p_acc,
                                     start=True, stop=False)
                    nc.tensor.matmul(denB[:, 384:512], ones128,
                                     p_lastpt[:, 384:512],
                                     start=False, stop=True)
                    rcdB = rcdp.tile([128, 512], F32)
                    nc.vector.reciprocal_approx_fast(rcdB, denB)
                    nc.vector.tensor_tensor(OT[:, p_h, p_tsl], p_ot, rcdB,
                                            op=ALU.mult)

                def scores(h, kv, qc, kp, spool):
                    kt0 = 2 * kp
                    st_pair = spool.tile([128, 2, 512], F32, tag="st")
                    pt_pair = ptp.tile([128, 2, 512], BF16)
                    if kt0 + 1 < 4 * qc:          # full pair
                        for half in (0, 1):
                            kt = kt0 + half
                            nc.tensor.matmul(
                                st_pair[:, half, :],
                                KTt[:, kv, kt * 128:(kt + 1) * 128],
                                QT[:, h, qc * 512:(qc + 1) * 512],
                                start=True, stop=True)
                        nc.scalar.activation(pt_pair, st_pair, AF.Exp)
                    else:                          # diagonal pair
                        d0 = kt0 - 4 * qc
                        for half in (0, 1):
                            kt = kt0 + half
                            d = kt - 4 * qc
                            vsl = slice(d * 128, 512)
                            nc.tensor.matmul(
                                st_pair[:, half, vsl],
                                KTt[:, kv, kt * 128:(kt + 1) * 128],
                                QT[:, h, qc * 512 + d * 128:(qc + 1) * 512],
                                start=True, stop=True)
                            if d0 == 0 and half == 1:
                                nc.scalar.activation(pt_pair, st_pair, AF.Exp)
                            elif d0 > 0:
                                nc.scalar.activation(
                                    pt_pair[:, half, vsl],
                                    st_pair[:, half, vsl], AF.Exp)
                            if d0 > 0 or half == 1:
                                for hh in ((0, 1) if (d0 == 0 and half == 1)
                                           else (half,)):
                                    dd = kt0 + hh - 4 * qc
                                    nc.gpsimd.tensor_tensor(
                                        pt_pair[:, hh, dd * 128:(dd + 1) * 128],
                                        pt_pair[:, hh, dd * 128:(dd + 1) * 128],
                                        cmask[:, dd, :], op=ALU.mult)
                    return pt_pair

                def consume(h, kv, qc, kp, pt_pair, otps, acc, last_kt):
                    for half in (0, 1):
                        kt = 2 * kp + half
                        d = kt - 4 * qc
                        vsl = slice(max(d, 0) * 128, 512)
                        ptv = pt_pair[:, half, :]
                        nc.tensor.matmul(
                            otps[:, vsl], Vn[:, kv, kt, :], ptv[:, vsl],
                            start=(kt == 0), stop=(kt == last_kt))
                        if kt == 0 and qc == 0:
                            nc.vector.tensor_copy(acc, ptv)
                        elif kt == 1 and qc > 0:
                            nc.vector.tensor_tensor(
                                acc, pt_pair[:, 0, :], pt_pair[:, 1, :],
                                op=ALU.add)
                        elif (kt > 1 or qc == 0) and kt < last_kt:
                            nc.vector.tensor_tensor(
                                acc[:, vsl], acc[:, vsl], ptv[:, vsl],
                                op=ALU.add)
                    return pt_pair[:, 1, :]

                def stage(qc, spool):
                    """Generator: attention stage qc (all heads), one
                    emission unit per yield."""
                    tsl = slice(qc * 512, (qc + 1) * 512)
                    nk = 4 * qc + 4
                    last_kt = nk - 1
                    npairs = nk // 2
                    for h in range(QH):
                        kv = h // 4
                        otps = pot.tile([128, 512], F32,
                                        name=f"ot{h}_{qc}", tag="ot")
                        acc = accp.tile([128, 512], BF16)
                        pts = [scores(h, kv, qc, 0, spool)]
                        yield
                        last_pt = None
                        for kp in range(npairs):
                            if kp + 1 < npairs:
                                pts.append(scores(h, kv, qc, kp + 1, spool))
                                yield
                            if kp == 0:
                                flush_pending()
                                yield
                            last_pt = consume(h, kv, qc, kp, pts[kp],
                                              otps, acc, last_kt)
                            yield
                        pending[0] = (acc, last_pt, otps, h, tsl)

                def vtrans_unit(kv_, kt_):
                    def f():
                        pt_ = pdb.tile([128, 128], BF16,
                                       name=f"tv{kv_}_{kt_}", tag="db")
                        nc.tensor.transpose(
                            pt_, VT[:, kv_, kt_ * 128:(kt_ + 1) * 128], ident)
                        nc.scalar.copy(Vn[:, kv_, kt_, :], pt_)
                    return f

                # ---------------- merged proj + stages 0..2 ----------------
                import itertools
                from collections import deque
                with tc.tile_pool(name="xt", bufs=2) as xtp, \
                     tc.tile_pool(name="wt", bufs=6) as wtp, \
                     tc.tile_pool(name="pproj", bufs=4, space="PSUM") as ppj:
                    xview = xT_d.ap().rearrange("(c p) t -> p c t", p=128)
                    sgen = None
                    carry = deque()   # prev chunk's KTt ropes + vtrans
                    for qc in range(QC):
                        tsl = slice(qc * 512, (qc + 1) * 512)
                        xt = xtp.tile([128, CCH, 512], BF16)
                        for piece in range(4):
                            csl = slice(piece * 8, (piece + 1) * 8)
                            nc.scalar.dma_start(xt[:, csl, :],
                                                xview[:, csl, tsl])
                        if qc == 0:
                            nc.scalar.dma_start(cosT, cos_d.ap())
                            nc.scalar.dma_start(sinT, sin_d.ap())
                            nc.scalar.dma_start(prot, prot_d.ap())
                            nc.scalar.dma_start(cmask, cmask_d.ap())
                        for p in range(3):
                            psums = [ppj.tile([128, 512], F32,
                                              name=f"pj{qc}_{p}_{o}",
                                              tag="pj") for o in range(4)]
                            for c in range(CCH):
                                wt = wtp.tile([128, 512], BF16)
                                nc.sync.dma_start(wt, wqkv_d.ap()[c, p])
                                for o in range(4):
                                    nc.tensor.matmul(
                                        psums[o], wt[:, o * 128:(o + 1) * 128],
                                        xt[:, c, :], start=(c == 0),
                                        stop=(c == CCH - 1))
                                # drip background work between proj matmuls:
                                # first leftover vtrans/K-ropes, then the
                                # interleaved attention stage, plus this
                                # chunk's Q ropes once their pass is done
                                if carry:
                                    carry.popleft()()
                                elif sgen is not None:
                                    next(sgen, None)
                                if p >= 1 and c % 8 == 4:
                                    hq = 4 * (p - 1) + c // 8
                                    rope_chunk(QT[:, hq, :], qc)
                            for o in range(4):
                                oi = 4 * p + o
                                if oi < 8:
                                    dst = QT[:, oi, tsl]
                                elif oi < 10:
                                    dst = KTt[:, oi - 8, tsl]
                                else:
                                    dst = VT[:, oi - 10, tsl]
                                if o % 2 == 0:
                                    nc.scalar.copy(dst, psums[o])
                                else:
                                    nc.vector.tensor_copy(dst, psums[o])
                            if sgen is not None:
                                next(sgen, None)
                        carry.extend(
                            [lambda q=qc: rope_chunk(KTt[:, 0, :], q),
                             lambda q=qc: rope_chunk(KTt[:, 1, :], q)] +
                            [vtrans_unit(kv_, kt_) for kv_ in range(KV)
                             for kt_ in range(4 * qc, 4 * qc + 4)])
                        if qc < 3:
                            nxt = stage(qc, pst)
                            sgen = nxt if sgen is None \
                                else itertools.chain(sgen, nxt)

                    # drain whatever of stages 0..2 is left, interleaved
                    # with the leftover chunk-3 vtrans/K-ropes
                    while carry:
                        carry.popleft()()
                        if sgen is not None:
                            next(sgen, None)
                    if sgen is not None:
                        for _ in sgen:
                            pass

                # wo streams in while stage 3 runs
                wo_pool = tc.alloc_tile_pool(name="wop", bufs=1)
                wo_t = wo_pool.tile([128, QH, C], BF16)
                nc.sync.dma_start(wo_t,
                                  wo_d.ap().rearrange("(h p) n -> p h n",
                                                      p=128))

                # ---------------- stage 3, with extra score-pair bufs ------
                pst3 = tc.alloc_tile_pool(name="pst3", bufs=2, space="PSUM")
                for _ in stage(3, pst3):
                    pass
                flush_pending()
                pst3.release()

                # ---------------- o_proj partial: O @ wo_slice ------------
                with tc.tile_pool(name="pout", bufs=4, space="PSUM") as outp, \
                     tc.tile_pool(name="ostg", bufs=6) as stgp:
                    for tt in range(TT):
                        psl = slice(tt * 128, (tt + 1) * 128)
                        for n in range(8):
                            nsl = slice(n * 512, (n + 1) * 512)
                            ps = outp.tile([128, 512], F32)
                            for h in range(QH):
                                nc.tensor.matmul(ps, OT[:, h, psl],
                                                 wo_t[:, h, nsl],
                                                 start=(h == 0),
                                                 stop=(h == QH - 1))
                            stg = stgp.tile([128, 512], F32)
                            nc.scalar.copy(stg, ps)
                            nc.sync.dma_start(out_d.ap()[psl, nsl], stg)

                wo_pool.release()

    nc.compile()
    return nc


def _host_prep(x, wq, wk, wv, wo):
    bf = ml_dtypes.bfloat16
    scale = HD ** -0.5

    inv_freq = 1.0 / (THETA ** (np.arange(0, HD, 2, dtype=np.float32) / HD))
    t = np.arange(T, dtype=np.float32)
    freqs = np.outer(t, inv_freq)
    emb = np.concatenate([freqs, freqs], -1)
    cosT = np.ascontiguousarray(np.cos(emb).T).astype(bf)
    sinT = np.ascontiguousarray(np.sin(emb).T).astype(bf)

    protT = np.zeros((128, 128), np.float32)
    for i in range(64):
        protT[i + 64, i] = -1.0
        protT[i, i + 64] = 1.0
    protT = protT.astype(bf)

    ident = np.eye(128, dtype=np.float32).astype(bf)

    kl = np.arange(128)[:, None]
    ql = np.arange(128)[None, :]
    blk = (ql >= kl).astype(np.float32)
    cmask01 = np.ascontiguousarray(
        np.stack([blk] * 4, axis=1).astype(bf))    # [128, 4, 128]

    xT = []
    for b in range(B):
        xT.append(np.ascontiguousarray(x[b].astype(bf).T))

    wqkv, wob = [], []
    for g in range(4):
        q_s = (wq[:, g * 1024:(g + 1) * 1024] * scale).astype(bf)
        k_s = wk[:, g * 256:(g + 1) * 256].astype(bf)
        v_s = wv[:, g * 256:(g + 1) * 256].astype(bf)
        wall = np.concatenate([q_s, k_s, v_s], axis=1)       # [C, 1536]
        wall = wall.reshape(CCH, 128, 3, 512).transpose(0, 2, 1, 3)
        wqkv.append(np.ascontiguousarray(wall))              # [32, 3, 128, 512]
        wob.append(np.ascontiguousarray(
            wo[g * 1024:(g + 1) * 1024, :].astype(bf)))

    in_maps = []
    for core in range(NCORES):
        b, g = core // 4, core % 4
        in_maps.append({
            "xT": xT[b], "wqkv": wqkv[g], "wo": wob[g],
            "cosT": cosT, "sinT": sinT, "protT": protT,
            "ident": ident, "cmask01": cmask01,
        })
    return in_maps


def kernel(x, wq, wk, wv, wo, _trace=False, _tmpdir=None):
    if "nc" not in _CACHE:
        _CACHE["nc"] = _build_nc()
    nc = _CACHE["nc"]

    in_maps = _host_prep(x, wq, wk, wv, wo)
    res = run_bass_kernel_spmd(nc, in_maps, core_ids=list(range(NCORES)),
                               trace=_trace, tmpdir=_tmpdir)
    _CACHE["last_results"] = res

    out = np.zeros((B, T, C), np.float32)
    for core in range(NCORES):
        out[core // 4] += res.results[core]["out"]
    return out


# revision 11
# speedup vs baseline: 1.3880x; 1.0250x over previous
"""GQA attention kernel for 8 TRN2 NeuronCores.

Problem: B=2, T=2048, C=4096, NH=32 q-heads, NKV=8 kv-heads, HD=128,
RoPE (theta=1e4), causal, f32 I/O.

Sharding: core = (batch b, kv-head-group g): b = core//4, g = core%4.

v5: the projection chunk loop and the attention "stages" are emitted
interleaved: attention stage qc (all 8 heads' scores/PV/softmax for
q-chunk qc, k<=qc) rides inside proj chunk qc+1's contraction loop, so
the softmax exp (ACT engine, ~180us total) hides under the PE-bound
projection stream instead of serializing after it. Stage 3 runs
post-proj with an extra score-pair pool. Flipped PV + broadcast
denominator as in v4 (see kernel_v4.py docstring).
"""

import sys

sys.path.insert(0, "/opt/trn_rl_repo")

import numpy as np
import ml_dtypes

import concourse.bass as bass
import concourse.bacc as bacc
import concourse.mybir as mybir
import concourse.tile as tile
from concourse.bass_utils import run_bass_kernel_spmd

BF16 = mybir.dt.bfloat16
F32 = mybir.dt.float32
AF = mybir.ActivationFunctionType
ALU = mybir.AluOpType

B, T, C = 2, 2048, 4096
NH, NKV, HD = 32, 8, 128
THETA = 10000.0
NCORES = 8

QH = 8
KV = 2
QC = 4
KT = 16
TT = 16
CCH = 32

_CACHE = {}


def _build_nc():
    nc = bacc.Bacc("TRN2", target_bir_lowering=False, debug=False,
                   enable_asserts=False, num_devices=NCORES)

    xT_d = nc.dram_tensor("xT", [C, T], BF16, kind="ExternalInput")
    wqkv_d = nc.dram_tensor("wqkv", [CCH, 3, 128, 512], BF16, kind="ExternalInput")
    wo_d = nc.dram_tensor("wo", [QH * HD, C], BF16, kind="ExternalInput")
    cos_d = nc.dram_tensor("cosT", [128, T], BF16, kind="ExternalInput")
    sin_d = nc.dram_tensor("sinT", [128, T], BF16, kind="ExternalInput")
    prot_d = nc.dram_tensor("protT", [128, 128], BF16, kind="ExternalInput")
    ident_d = nc.dram_tensor("ident", [128, 128], BF16, kind="ExternalInput")
    cmask_d = nc.dram_tensor("cmask01", [128, 4, 128], BF16, kind="ExternalInput")
    out_d = nc.dram_tensor("out", [T, C], F32, kind="ExternalOutput")

    with tile.TileContext(nc) as tc:
        with tc.tile_pool(name="persist", bufs=1) as pp:
            ident = pp.tile([128, 128], BF16)
            nc.sync.dma_start(ident, ident_d.ap())
            cosT = pp.tile([128, T], BF16)
            sinT = pp.tile([128, T], BF16)
            prot = pp.tile([128, 128], BF16)
            cmask = pp.tile([128, 4, 128], BF16)
            ones128 = pp.tile([128, 128], BF16)
            warm = pp.tile([128, 512], BF16)
            nc.vector.memset(ones128, 1.0)
            nc.vector.memset(warm, 0.25)

            with tc.tile_pool(name="pwarm", bufs=2, space="PSUM") as pwp:
                for w in range(24):
                    wps = pwp.tile([128, 512], F32, name=f"warm{w}", tag="warm")
                    nc.tensor.matmul(wps, warm[:, 0:128], warm,
                                     start=True, stop=True)

            QT = pp.tile([128, QH, T], BF16)
            KTt = pp.tile([128, KV, T], BF16)
            VT = pp.tile([128, KV, T], BF16)
            OT = pp.tile([128, QH, T], BF16)
            Vn = pp.tile([128, KV, KT, 128], BF16)

            # attention pools live for both the merged phase and stage 3
            with tc.tile_pool(name="pst", bufs=1, space="PSUM") as pst, \
                 tc.tile_pool(name="pot", bufs=1, space="PSUM") as pot, \
                 tc.tile_pool(name="pdb", bufs=1, space="PSUM") as pdb, \
                 tc.tile_pool(name="pt", bufs=3) as ptp, \
                 tc.tile_pool(name="acc", bufs=2) as accp, \
                 tc.tile_pool(name="rcd", bufs=2) as rcdp, \
                 tc.tile_pool(name="ropes", bufs=3) as rsp:

                def rope_chunk(src, rqc):
                    rsl = slice(rqc * 512, (rqc + 1) * 512)
                    ps = pdb.tile([128, 512], F32, name=f"rot{rqc}", tag="db")
                    nc.tensor.matmul(ps, prot, src[:, rsl], start=True,
                                     stop=True)
                    rs = rsp.tile([128, 512], BF16, name=f"rs{rqc}", tag="rs")
                    nc.vector.tensor_tensor(rs, ps, sinT[:, rsl], op=ALU.mult)
                    nc.vector.tensor_tensor(src[:, rsl], src[:, rsl],
                                            cosT[:, rsl], op=ALU.mult)
                    nc.vector.tensor_tensor(src[:, rsl], src[:, rsl], rs,
                                            op=ALU.add)

                pending = [None]

                def flush_pending():
                    if pending[0] is None:
                        return
                    p_acc, p_lastpt, p_ot, p_h, p_tsl = pending[0]
                    pending[0] = None
                    denB = pdb.tile([128, 512], F32, tag="db")
                    nc.tensor.matmul(denB, ones128, p_acc,
                                     start=True, stop=False)
                    nc.tensor.matmul(denB[:, 384:512], ones128,
                                     p_lastpt[:, 384:512],
                                     start=False, stop=True)
                    rcdB = rcdp.tile([128, 512], F32)
                    nc.vector.reciprocal_approx_fast(rcdB, denB)
                    nc.vector.tensor_tensor(OT[:, p_h, p_tsl], p_ot, rcdB,
                                            op=ALU.mult)

                def scores(h, kv, qc, kp, spool):
                    kt0 = 2 * kp
                    st_pair = spool.tile([128, 2, 512], F32, tag="st")
                    pt_pair = ptp.tile([128, 2, 512], BF16)
                    if kt0 + 1 < 4 * qc:          # full pair
                        for half in (0, 1):
                            kt = kt0 + half
                            nc.tensor.matmul(
                                st_pair[:, half, :],
                                KTt[:, kv, kt * 128:(kt + 1) * 128],
                                QT[:, h, qc * 512:(qc + 1) * 512],
                                start=True, stop=True)
                        nc.scalar.activation(pt_pair, st_pair, AF.Exp)
                    else:                          # diagonal pair
                        d0 = kt0 - 4 * qc
                        for half in (0, 1):
                            kt = kt0 + half
                            d = kt - 4 * qc
                            vsl = slice(d * 128, 512)
                            nc.tensor.matmul(
                                st_pair[:, half, vsl],
                                KTt[:, kv, kt * 128:(kt + 1) * 128],
                                QT[:, h, qc * 512 + d * 128:(qc + 1) * 512],
                                start=True, stop=True)
                            if d0 == 0 and half == 1:
                                nc.scalar.activation(pt_pair, st_pair, AF.Exp)
                            elif d0 > 0:
                                nc.scalar.activation(
                                    pt_pair[:, half, vsl],
                                    st_pair[:, half, vsl], AF.Exp)
                            if d0 > 0 or half == 1:
                                for hh in ((0, 1) if (d0 == 0 and half == 1)
                                           else (half,)):
                                    dd = kt0 + hh - 4 * qc
                                    nc.gpsimd.tensor_tensor(
                                        pt_pair[:, hh, dd * 128:(dd + 1) * 128],
                                        pt_pair[:, hh, dd * 128:(dd + 1) * 128],
                                        cmask[:, dd, :], op=ALU.mult)
                    return pt_pair

                def consume(h, kv, qc, kp, pt_pair, otps, acc, last_kt):
                    for half in (0, 1):
                        kt = 2 * kp + half
                        d = kt - 4 * qc
                        vsl = slice(max(d, 0) * 128, 512)
                        ptv = pt_pair[:, half, :]
                        nc.tensor.matmul(
                            otps[:, vsl], Vn[:, kv, kt, :], ptv[:, vsl],
                            start=(kt == 0), stop=(kt == last_kt))
                        if kt == 0 and qc == 0:
                            nc.vector.tensor_copy(acc, ptv)
                        elif kt == 1 and qc > 0:
                            nc.vector.tensor_tensor(
                                acc, pt_pair[:, 0, :], pt_pair[:, 1, :],
                                op=ALU.add)
                        elif (kt > 1 or qc == 0) and kt < last_kt:
                            nc.vector.tensor_tensor(
                                acc[:, vsl], acc[:, vsl], ptv[:, vsl],
                                op=ALU.add)
                    return pt_pair[:, 1, :]

                def stage(qc, spool):
                    """Generator: attention stage qc (all heads), one
                    emission unit per yield."""
                    tsl = slice(qc * 512, (qc + 1) * 512)
                    nk = 4 * qc + 4
                    last_kt = nk - 1
                    npairs = nk // 2
                    for h in range(QH):
                        kv = h // 4
                        otps = pot.tile([128, 512], F32,
                                        name=f"ot{h}_{qc}", tag="ot")
                        acc = accp.tile([128, 512], BF16)
                        pts = [scores(h, kv, qc, 0, spool)]
                        yield
                        last_pt = None
                        for kp in range(npairs):
                            if kp + 1 < npairs:
                                pts.append(scores(h, kv, qc, kp + 1, spool))
                                yield
                            if kp == 0:
                                flush_pending()
                                yield
                            last_pt = consume(h, kv, qc, kp, pts[kp],
                                              otps, acc, last_kt)
                            yield
                        pending[0] = (acc, last_pt, otps, h, tsl)

                def vtrans_unit(kv_, kt_):
                    def f():
                        pt_ = pdb.tile([128, 128], BF16,
                                       name=f"tv{kv_}_{kt_}", tag="db")
                        nc.tensor.transpose(
                            pt_, VT[:, kv_, kt_ * 128:(kt_ + 1) * 128], ident)
                        nc.scalar.copy(Vn[:, kv_, kt_, :], pt_)
                    return f

                # ---------------- merged proj + stages 0..2 ----------------
                import itertools
                from collections import deque
                with tc.tile_pool(name="xt", bufs=2) as xtp, \
                     tc.tile_pool(name="wt", bufs=6) as wtp, \
                     tc.tile_pool(name="pproj", bufs=4, space="PSUM") as ppj:
                    xview = xT_d.ap().rearrange("(c p) t -> p c t", p=128)
                    sgen = None
                    carry = deque()   # prev chunk's KTt ropes + vtrans
                    for qc in range(QC):
                        tsl = slice(qc * 512, (qc + 1) * 512)
                        xt = xtp.tile([128, CCH, 512], BF16)
                        bounds = [0, 2, 4, 8, 16, 32] if qc == 0 \
                            else [0, 8, 16, 24, 32]
                        for lo, hi in zip(bounds, bounds[1:]):
                            csl = slice(lo, hi)
                            nc.scalar.dma_start(xt[:, csl, :],
                                                xview[:, csl, tsl])
                        if qc == 0:
                            nc.scalar.dma_start(cosT, cos_d.ap())
                            nc.scalar.dma_start(sinT, sin_d.ap())
                            nc.scalar.dma_start(prot, prot_d.ap())
                            nc.scalar.dma_start(cmask, cmask_d.ap())
                        for p in range(3):
                            psums = [ppj.tile([128, 512], F32,
                                              name=f"pj{qc}_{p}_{o}",
                                              tag="pj") for o in range(4)]
                            for c in range(CCH):
                                wt = wtp.tile([128, 512], BF16)
                                nc.sync.dma_start(wt, wqkv_d.ap()[c, p])
                                for o in range(4):
                                    nc.tensor.matmul(
                                        psums[o], wt[:, o * 128:(o + 1) * 128],
                                        xt[:, c, :], start=(c == 0),
                                        stop=(c == CCH - 1))
                                # drip background work between proj matmuls:
                                # first leftover vtrans/K-ropes, then the
                                # interleaved attention stage, plus this
                                # chunk's Q ropes once their pass is done
                                if carry:
                                    carry.popleft()()
                                elif sgen is not None:
                                    next(sgen, None)
                                if p >= 1 and c % 8 == 4:
                                    hq = 4 * (p - 1) + c // 8
                                    rope_chunk(QT[:, hq, :], qc)
                            for o in range(4):
                                oi = 4 * p + o
                                if oi < 8:
                                    dst = QT[:, oi, tsl]
                                elif oi < 10:
                                    dst = KTt[:, oi - 8, tsl]
                                else:
                                    dst = VT[:, oi - 10, tsl]
                                if o % 2 == 0:
                                    nc.scalar.copy(dst, psums[o])
                                else:
                                    nc.vector.tensor_copy(dst, psums[o])
                            if sgen is not None:
                                next(sgen, None)
                        carry.extend(
                            [lambda q=qc: rope_chunk(KTt[:, 0, :], q),
                             lambda q=qc: rope_chunk(KTt[:, 1, :], q)] +
                            [vtrans_unit(kv_, kt_) for kv_ in range(KV)
                             for kt_ in range(4 * qc, 4 * qc + 4)])
                        if qc < 3:
                            nxt = stage(qc, pst)
                            sgen = nxt if sgen is None \
                                else itertools.chain(sgen, nxt)

                    # drain whatever of stages 0..2 is left, interleaved
                    # with the leftover chunk-3 vtrans/K-ropes
                    while carry:
                        carry.popleft()()
                        if sgen is not None:
                            next(sgen, None)
                    if sgen is not None:
                        for _ in sgen:
                            pass

                # wo streams in n-block slices so o_proj's first tiles
                # don't wait for the whole 8.4MB
                wo_pool = tc.alloc_tile_pool(name="wop", bufs=1)
                wo_t = wo_pool.tile([128, QH, C], BF16)
                wo_view = wo_d.ap().rearrange("(h p) n -> p h n", p=128)
                for n in range(8):
                    nsl = slice(n * 512, (n + 1) * 512)
                    nc.sync.dma_start(wo_t[:, :, nsl], wo_view[:, :, nsl])

                # ------- o_proj tt 0..11 with stage 3 interleaved ---------
                # (tokens 0..1535 only need stages 0..2, so stage 3's exp
                # hides under the PE-bound o_proj stream)
                pst3 = tc.alloc_tile_pool(name="pst3", bufs=1, space="PSUM")
                s3 = stage(3, pst3)

                with tc.tile_pool(name="pout", bufs=2, space="PSUM") as outp, \
                     tc.tile_pool(name="ostg", bufs=6) as stgp:
                    def oproj_tile(tt, n):
                        psl = slice(tt * 128, (tt + 1) * 128)
                        nsl = slice(n * 512, (n + 1) * 512)
                        ps = outp.tile([128, 512], F32)
                        for h in range(QH):
                            nc.tensor.matmul(ps, OT[:, h, psl],
                                             wo_t[:, h, nsl],
                                             start=(h == 0),
                                             stop=(h == QH - 1))
                        stg = stgp.tile([128, 512], F32)
                        nc.scalar.copy(stg, ps)
                        nc.sync.dma_start(out_d.ap()[psl, nsl], stg)

                    for tt in range(12):
                        for n in range(8):
                            oproj_tile(tt, n)
                            next(s3, None)
                            if tt % 2 == 0:
                                next(s3, None)
                    for _ in s3:
                        pass
                    flush_pending()
                    for tt in range(12, TT):
                        for n in range(8):
                            oproj_tile(tt, n)

                pst3.release()
                wo_pool.release()

    nc.compile()
    return nc


def _host_prep(x, wq, wk, wv, wo):
    bf = ml_dtypes.bfloat16
    scale = HD ** -0.5

    inv_freq = 1.0 / (THETA ** (np.arange(0, HD, 2, dtype=np.float32) / HD))
    t = np.arange(T, dtype=np.float32)
    freqs = np.outer(t, inv_freq)
    emb = np.concatenate([freqs, freqs], -1)
    cosT = np.ascontiguousarray(np.cos(emb).T).astype(bf)
    sinT = np.ascontiguousarray(np.sin(emb).T).astype(bf)

    protT = np.zeros((128, 128), np.float32)
    for i in range(64):
        protT[i + 64, i] = -1.0
        protT[i, i + 64] = 1.0
    protT = protT.astype(bf)

    ident = np.eye(128, dtype=np.float32).astype(bf)

    kl = np.arange(128)[:, None]
    ql = np.arange(128)[None, :]
    blk = (ql >= kl).astype(np.float32)
    cmask01 = np.ascontiguousarray(
        np.stack([blk] * 4, axis=1).astype(bf))    # [128, 4, 128]

    xT = []
    for b in range(B):
        xT.append(np.ascontiguousarray(x[b].astype(bf).T))

    wqkv, wob = [], []
    for g in range(4):
        q_s = (wq[:, g * 1024:(g + 1) * 1024] * scale).astype(bf)
        k_s = wk[:, g * 256:(g + 1) * 256].astype(bf)
        v_s = wv[:, g * 256:(g + 1) * 256].astype(bf)
        wall = np.concatenate([q_s, k_s, v_s], axis=1)       # [C, 1536]
        wall = wall.reshape(CCH, 128, 3, 512).transpose(0, 2, 1, 3)
        wqkv.append(np.ascontiguousarray(wall))              # [32, 3, 128, 512]
        wob.append(np.ascontiguousarray(
            wo[g * 1024:(g + 1) * 1024, :].astype(bf)))

    in_maps = []
    for core in range(NCORES):
        b, g = core // 4, core % 4
        in_maps.append({
            "xT": xT[b], "wqkv": wqkv[g], "wo": wob[g],
            "cosT": cosT, "sinT": sinT, "protT": protT,
            "ident": ident, "cmask01": cmask01,
        })
    return in_maps


def kernel(x, wq, wk, wv, wo, _trace=False, _tmpdir=None):
    if "nc" not in _CACHE:
        _CACHE["nc"] = _build_nc()
    nc = _CACHE["nc"]

    in_maps = _host_prep(x, wq, wk, wv, wo)
    res = run_bass_kernel_spmd(nc, in_maps, core_ids=list(range(NCORES)),
                               trace=_trace, tmpdir=_tmpdir)
    _CACHE["last_results"] = res

    out = np.zeros((B, T, C), np.float32)
    for core in range(NCORES):
        out[core // 4] += res.results[core]["out"]
    return out


# revision 12
# speedup vs baseline: 1.4112x; 1.0167x over previous
"""GQA attention kernel for 8 TRN2 NeuronCores.

Problem: B=2, T=2048, C=4096, NH=32 q-heads, NKV=8 kv-heads, HD=128,
RoPE (theta=1e4), causal, f32 I/O.

Sharding: core = (batch b, kv-head-group g): b = core//4, g = core%4.

v5: the projection chunk loop and the attention "stages" are emitted
interleaved: attention stage qc (all 8 heads' scores/PV/softmax for
q-chunk qc, k<=qc) rides inside proj chunk qc+1's contraction loop, so
the softmax exp (ACT engine, ~180us total) hides under the PE-bound
projection stream instead of serializing after it. Stage 3 runs
post-proj with an extra score-pair pool. Flipped PV + broadcast
denominator as in v4 (see kernel_v4.py docstring).
"""

import sys

sys.path.insert(0, "/opt/trn_rl_repo")

import numpy as np
import ml_dtypes

import concourse.bass as bass
import concourse.bacc as bacc
import concourse.mybir as mybir
import concourse.tile as tile
from concourse.bass_utils import run_bass_kernel_spmd

BF16 = mybir.dt.bfloat16
F32 = mybir.dt.float32
AF = mybir.ActivationFunctionType
ALU = mybir.AluOpType

B, T, C = 2, 2048, 4096
NH, NKV, HD = 32, 8, 128
THETA = 10000.0
NCORES = 8

QH = 8
KV = 2
QC = 4
KT = 16
TT = 16
CCH = 32

_CACHE = {}


def _build_nc():
    nc = bacc.Bacc("TRN2", target_bir_lowering=False, debug=False,
                   enable_asserts=False, num_devices=NCORES)

    xT_d = nc.dram_tensor("xT", [C, T], BF16, kind="ExternalInput")
    wqkv_d = nc.dram_tensor("wqkv", [CCH, 3, 128, 512], BF16, kind="ExternalInput")
    wo_d = nc.dram_tensor("wo", [QH * HD, C], BF16, kind="ExternalInput")
    cos_d = nc.dram_tensor("cosT", [128, T], BF16, kind="ExternalInput")
    sin_d = nc.dram_tensor("sinT", [128, T], BF16, kind="ExternalInput")
    prot_d = nc.dram_tensor("protT", [128, 128], BF16, kind="ExternalInput")
    ident_d = nc.dram_tensor("ident", [128, 128], BF16, kind="ExternalInput")
    cmask_d = nc.dram_tensor("cmask01", [128, 4, 128], BF16, kind="ExternalInput")
    out_d = nc.dram_tensor("out", [T, C], F32, kind="ExternalOutput")

    with tile.TileContext(nc) as tc:
        with tc.tile_pool(name="persist", bufs=1) as pp:
            ident = pp.tile([128, 128], BF16)
            nc.sync.dma_start(ident, ident_d.ap())
            cosT = pp.tile([128, T], BF16)
            sinT = pp.tile([128, T], BF16)
            prot = pp.tile([128, 128], BF16)
            cmask = pp.tile([128, 4, 128], BF16)
            ones128 = pp.tile([128, 128], BF16)
            warm = pp.tile([128, 512], BF16)
            nc.vector.memset(ones128, 1.0)
            nc.vector.memset(warm, 0.25)

            with tc.tile_pool(name="pwarm", bufs=2, space="PSUM") as pwp:
                for w in range(24):
                    wps = pwp.tile([128, 512], F32, name=f"warm{w}", tag="warm")
                    nc.tensor.matmul(wps, warm[:, 0:128], warm,
                                     start=True, stop=True)

            QT = pp.tile([128, QH, T], BF16)
            KTt = pp.tile([128, KV, T], BF16)
            VT = pp.tile([128, KV, T], BF16)
            OT = pp.tile([128, QH, T], BF16)
            Vn = pp.tile([128, KV, KT, 128], BF16)

            # attention pools live for both the merged phase and stage 3
            with tc.tile_pool(name="pst", bufs=1, space="PSUM") as pst, \
                 tc.tile_pool(name="pot", bufs=1, space="PSUM") as pot, \
                 tc.tile_pool(name="pdb", bufs=1, space="PSUM") as pdb, \
                 tc.tile_pool(name="pt", bufs=4) as ptp, \
                 tc.tile_pool(name="acc", bufs=2) as accp, \
                 tc.tile_pool(name="rcd", bufs=2) as rcdp, \
                 tc.tile_pool(name="ropes", bufs=3) as rsp:

                def rope_chunk(src, rqc):
                    rsl = slice(rqc * 512, (rqc + 1) * 512)
                    ps = pdb.tile([128, 512], F32, name=f"rot{rqc}", tag="db")
                    nc.tensor.matmul(ps, prot, src[:, rsl], start=True,
                                     stop=True)
                    rs = rsp.tile([128, 512], BF16, name=f"rs{rqc}", tag="rs")
                    nc.vector.tensor_tensor(rs, ps, sinT[:, rsl], op=ALU.mult)
                    nc.vector.tensor_tensor(src[:, rsl], src[:, rsl],
                                            cosT[:, rsl], op=ALU.mult)
                    nc.vector.tensor_tensor(src[:, rsl], src[:, rsl], rs,
                                            op=ALU.add)

                pending = [None]

                def flush_pending():
                    if pending[0] is None:
                        return
                    p_acc, p_lastpt, p_ot, p_h, p_tsl = pending[0]
                    pending[0] = None
                    denB = pdb.tile([128, 512], F32, tag="db")
                    nc.tensor.matmul(denB, ones128, p_acc,
                                     start=True, stop=False)
                    nc.tensor.matmul(denB[:, 384:512], ones128,
                                     p_lastpt[:, 384:512],
                                     start=False, stop=True)
                    rcdB = rcdp.tile([128, 512], F32)
                    nc.vector.reciprocal_approx_fast(rcdB, denB)
                    nc.vector.tensor_tensor(OT[:, p_h, p_tsl], p_ot, rcdB,
                                            op=ALU.mult)

                def scores(h, kv, qc, kp, spool):
                    kt0 = 2 * kp
                    st_pair = spool.tile([128, 2, 512], F32, tag="st")
                    pt_pair = ptp.tile([128, 2, 512], BF16)
                    if kt0 + 1 < 4 * qc:          # full pair
                        for half in (0, 1):
                            kt = kt0 + half
                            nc.tensor.matmul(
                                st_pair[:, half, :],
                                KTt[:, kv, kt * 128:(kt + 1) * 128],
                                QT[:, h, qc * 512:(qc + 1) * 512],
                                start=True, stop=True)
                        nc.scalar.activation(pt_pair, st_pair, AF.Exp)
                    else:                          # diagonal pair
                        d0 = kt0 - 4 * qc
                        for half in (0, 1):
                            kt = kt0 + half
                            d = kt - 4 * qc
                            vsl = slice(d * 128, 512)
                            nc.tensor.matmul(
                                st_pair[:, half, vsl],
                                KTt[:, kv, kt * 128:(kt + 1) * 128],
                                QT[:, h, qc * 512 + d * 128:(qc + 1) * 512],
                                start=True, stop=True)
                            if d0 == 0 and half == 1:
                                nc.scalar.activation(pt_pair, st_pair, AF.Exp)
                            elif d0 > 0:
                                nc.scalar.activation(
                                    pt_pair[:, half, vsl],
                                    st_pair[:, half, vsl], AF.Exp)
                            if d0 > 0 or half == 1:
                                for hh in ((0, 1) if (d0 == 0 and half == 1)
                                           else (half,)):
                                    dd = kt0 + hh - 4 * qc
                                    nc.gpsimd.tensor_tensor(
                                        pt_pair[:, hh, dd * 128:(dd + 1) * 128],
                                        pt_pair[:, hh, dd * 128:(dd + 1) * 128],
                                        cmask[:, dd, :], op=ALU.mult)
                    return pt_pair

                def consume(h, kv, qc, kp, pt_pair, otps, acc, last_kt):
                    for half in (0, 1):
                        kt = 2 * kp + half
                        d = kt - 4 * qc
                        vsl = slice(max(d, 0) * 128, 512)
                        ptv = pt_pair[:, half, :]
                        nc.tensor.matmul(
                            otps[:, vsl], Vn[:, kv, kt, :], ptv[:, vsl],
                            start=(kt == 0), stop=(kt == last_kt))
                        if kt == 0 and qc == 0:
                            nc.vector.tensor_copy(acc, ptv)
                        elif kt == 1 and qc > 0:
                            nc.vector.tensor_tensor(
                                acc, pt_pair[:, 0, :], pt_pair[:, 1, :],
                                op=ALU.add)
                        elif (kt > 1 or qc == 0) and kt < last_kt:
                            nc.vector.tensor_tensor(
                                acc[:, vsl], acc[:, vsl], ptv[:, vsl],
                                op=ALU.add)
                    return pt_pair[:, 1, :]

                def stage(qc, spool):
                    """Generator: attention stage qc (all heads), one
                    emission unit per yield."""
                    tsl = slice(qc * 512, (qc + 1) * 512)
                    nk = 4 * qc + 4
                    last_kt = nk - 1
                    npairs = nk // 2
                    for h in range(QH):
                        kv = h // 4
                        otps = pot.tile([128, 512], F32,
                                        name=f"ot{h}_{qc}", tag="ot")
                        acc = accp.tile([128, 512], BF16)
                        pts = [scores(h, kv, qc, 0, spool)]
                        yield
                        last_pt = None
                        for kp in range(npairs):
                            if kp + 1 < npairs:
                                pts.append(scores(h, kv, qc, kp + 1, spool))
                                yield
                            if kp == 0:
                                flush_pending()
                                yield
                            last_pt = consume(h, kv, qc, kp, pts[kp],
                                              otps, acc, last_kt)
                            yield
                        pending[0] = (acc, last_pt, otps, h, tsl)

                def vtrans_unit(kv_, kt_):
                    def f():
                        pt_ = pdb.tile([128, 128], BF16,
                                       name=f"tv{kv_}_{kt_}", tag="db")
                        nc.tensor.transpose(
                            pt_, VT[:, kv_, kt_ * 128:(kt_ + 1) * 128], ident)
                        nc.scalar.copy(Vn[:, kv_, kt_, :], pt_)
                    return f

                # ---------------- merged proj + stages 0..2 ----------------
                import itertools
                from collections import deque
                with tc.tile_pool(name="xt", bufs=2) as xtp, \
                     tc.tile_pool(name="wt", bufs=6) as wtp, \
                     tc.tile_pool(name="pproj", bufs=4, space="PSUM") as ppj:
                    xview = xT_d.ap().rearrange("(c p) t -> p c t", p=128)
                    sgen = None
                    carry = deque()   # prev chunk's KTt ropes + vtrans
                    for qc in range(QC):
                        tsl = slice(qc * 512, (qc + 1) * 512)
                        xt = xtp.tile([128, CCH, 512], BF16)
                        bounds = [0, 1, 2, 3, 4, 6, 8, 12, 16, 24, 32] \
                            if qc == 0 else [0, 8, 16, 24, 32]
                        for lo, hi in zip(bounds, bounds[1:]):
                            csl = slice(lo, hi)
                            nc.scalar.dma_start(xt[:, csl, :],
                                                xview[:, csl, tsl])
                            if qc == 0 and hi == 8:
                                # tables squeezed mid-stream: after the
                                # urgent first c-chunks, before the rope of
                                # chunk 0 needs them (~40us in)
                                nc.scalar.dma_start(cosT, cos_d.ap())
                                nc.scalar.dma_start(sinT, sin_d.ap())
                                nc.scalar.dma_start(prot, prot_d.ap())
                                nc.scalar.dma_start(cmask, cmask_d.ap())
                        for p in range(3):
                            psums = [ppj.tile([128, 512], F32,
                                              name=f"pj{qc}_{p}_{o}",
                                              tag="pj") for o in range(4)]
                            for c in range(CCH):
                                wt = wtp.tile([128, 512], BF16)
                                nc.sync.dma_start(wt, wqkv_d.ap()[c, p])
                                for o in range(4):
                                    nc.tensor.matmul(
                                        psums[o], wt[:, o * 128:(o + 1) * 128],
                                        xt[:, c, :], start=(c == 0),
                                        stop=(c == CCH - 1))
                                # drip background work between proj matmuls:
                                # first leftover vtrans/K-ropes, then the
                                # interleaved attention stage, plus this
                                # chunk's Q ropes once their pass is done
                                if carry:
                                    carry.popleft()()
                                elif sgen is not None:
                                    next(sgen, None)
                                if p >= 1 and c % 8 == 4:
                                    hq = 4 * (p - 1) + c // 8
                                    rope_chunk(QT[:, hq, :], qc)
                            for o in range(4):
                                oi = 4 * p + o
                                if oi < 8:
                                    dst = QT[:, oi, tsl]
                                elif oi < 10:
                                    dst = KTt[:, oi - 8, tsl]
                                else:
                                    dst = VT[:, oi - 10, tsl]
                                if o % 2 == 0:
                                    nc.scalar.copy(dst, psums[o])
                                else:
                                    nc.vector.tensor_copy(dst, psums[o])
                            if sgen is not None:
                                next(sgen, None)
                        carry.extend(
                            [lambda q=qc: rope_chunk(KTt[:, 0, :], q),
                             lambda q=qc: rope_chunk(KTt[:, 1, :], q)] +
                            [vtrans_unit(kv_, kt_) for kv_ in range(KV)
                             for kt_ in range(4 * qc, 4 * qc + 4)])
                        if qc < 3:
                            nxt = stage(qc, pst)
                            sgen = nxt if sgen is None \
                                else itertools.chain(sgen, nxt)

                    # drain whatever of stages 0..2 is left, interleaved
                    # with the leftover chunk-3 vtrans/K-ropes
                    while carry:
                        carry.popleft()()
                        if sgen is not None:
                            next(sgen, None)
                    if sgen is not None:
                        for _ in sgen:
                            pass

                # wo streams in n-block slices so o_proj's first tiles
                # don't wait for the whole 8.4MB
                wo_pool = tc.alloc_tile_pool(name="wop", bufs=1)
                wo_t = wo_pool.tile([128, QH, C], BF16)
                wo_view = wo_d.ap().rearrange("(h p) n -> p h n", p=128)
                for n in range(8):
                    nsl = slice(n * 512, (n + 1) * 512)
                    nc.sync.dma_start(wo_t[:, :, nsl], wo_view[:, :, nsl])

                # ------- o_proj tt 0..11 with stage 3 interleaved ---------
                # (tokens 0..1535 only need stages 0..2, so stage 3's exp
                # hides under the PE-bound o_proj stream)
                pst3 = tc.alloc_tile_pool(name="pst3", bufs=1, space="PSUM")
                s3 = stage(3, pst3)

                with tc.tile_pool(name="pout", bufs=2, space="PSUM") as outp, \
                     tc.tile_pool(name="ostg", bufs=6) as stgp:
                    def oproj_tile(tt, n):
                        psl = slice(tt * 128, (tt + 1) * 128)
                        nsl = slice(n * 512, (n + 1) * 512)
                        ps = outp.tile([128, 512], F32)
                        for h in range(QH):
                            nc.tensor.matmul(ps, OT[:, h, psl],
                                             wo_t[:, h, nsl],
                                             start=(h == 0),
                                             stop=(h == QH - 1))
                        stg = stgp.tile([128, 512], F32)
                        nc.scalar.copy(stg, ps)
                        if (tt + n) % 2 == 0:
                            nc.sync.dma_start(out_d.ap()[psl, nsl], stg)
                        else:
                            nc.scalar.dma_start(out_d.ap()[psl, nsl], stg)

                    for tt in range(12):
                        for n in range(8):
                            oproj_tile(tt, n)
                            next(s3, None)
                            if tt % 2 == 0:
                                next(s3, None)
                    for _ in s3:
                        pass
                    flush_pending()
                    for tt in range(12, TT):
                        for n in range(8):
                            oproj_tile(tt, n)

                pst3.release()
                wo_pool.release()

    nc.compile()
    return nc


def _host_prep(x, wq, wk, wv, wo):
    bf = ml_dtypes.bfloat16
    scale = HD ** -0.5

    inv_freq = 1.0 / (THETA ** (np.arange(0, HD, 2, dtype=np.float32) / HD))
    t = np.arange(T, dtype=np.float32)
    freqs = np.outer(t, inv_freq)
    emb = np.concatenate([freqs, freqs], -1)
    cosT = np.ascontiguousarray(np.cos(emb).T).astype(bf)
    sinT = np.ascontiguousarray(np.sin(emb).T).astype(bf)

    protT = np.zeros((128, 128), np.float32)
    for i in range(64):
        protT[i + 64, i] = -1.0
        protT[i, i + 64] = 1.0
    protT = protT.astype(bf)

    ident = np.eye(128, dtype=np.float32).astype(bf)

    kl = np.arange(128)[:, None]
    ql = np.arange(128)[None, :]
    blk = (ql >= kl).astype(np.float32)
    cmask01 = np.ascontiguousarray(
        np.stack([blk] * 4, axis=1).astype(bf))    # [128, 4, 128]

    xT = []
    for b in range(B):
        xT.append(np.ascontiguousarray(x[b].astype(bf).T))

    wqkv, wob = [], []
    for g in range(4):
        q_s = (wq[:, g * 1024:(g + 1) * 1024] * scale).astype(bf)
        k_s = wk[:, g * 256:(g + 1) * 256].astype(bf)
        v_s = wv[:, g * 256:(g + 1) * 256].astype(bf)
        wall = np.concatenate([q_s, k_s, v_s], axis=1)       # [C, 1536]
        wall = wall.reshape(CCH, 128, 3, 512).transpose(0, 2, 1, 3)
        wqkv.append(np.ascontiguousarray(wall))              # [32, 3, 128, 512]
        wob.append(np.ascontiguousarray(
            wo[g * 1024:(g + 1) * 1024, :].astype(bf)))

    in_maps = []
    for core in range(NCORES):
        b, g = core // 4, core % 4
        in_maps.append({
            "xT": xT[b], "wqkv": wqkv[g], "wo": wob[g],
            "cosT": cosT, "sinT": sinT, "protT": protT,
            "ident": ident, "cmask01": cmask01,
        })
    return in_maps


def kernel(x, wq, wk, wv, wo, _trace=False, _tmpdir=None):
    if "nc" not in _CACHE:
        _CACHE["nc"] = _build_nc()
    nc = _CACHE["nc"]

    in_maps = _host_prep(x, wq, wk, wv, wo)
    res = run_bass_kernel_spmd(nc, in_maps, core_ids=list(range(NCORES)),
                               trace=_trace, tmpdir=_tmpdir)
    _CACHE["last_results"] = res

    out = np.zeros((B, T, C), np.float32)
    for core in range(NCORES):
        out[core // 4] += res.results[core]["out"]
    return out
